# revision 10
# baseline (speedup 1.0000x reference)
"""Causal self-attention (B=1, T=4096, D=1024, H=16, dh=64) on 8 trn2 NeuronCores.

Sharding: tensor-parallel over heads — each core owns 2 of the 16 heads.
Per core: QKV projection (transposed activation layout), RoPE, causal
flash-style attention with transposed score tiles St[k,q] (so the AV matmul
needs no P transposes), softmax denominator via an appended ones-column in V,
out-projection against this core's W_out column slice -> partial output
[T, D] in bf16.  Host sums the 8 partials in f32.

v2: all matmul operands bf16 (fp32r ran at ~half PE rate on hw), Phase-B
software pipeline over a 4-slot PSUM score ring (S matmuls run 4 units
ahead of the AV matmuls so the exp on the scalar engine never stalls the
PE), softmax reciprocal computed in-lane at partition 64 + broadcast via
K=1 matmuls (replaces a DRAM round-trip), out-projection matmuls
interleaved into the AV drain window, bf16 output partials (halves the
output DMA), input/const DMAs spread across engines.
"""

import sys

sys.path.insert(0, "/opt/trn_rl_repo")

import numpy as np

import concourse.bass as bass
import concourse.tile as tile
from concourse import bacc, mybir
from concourse.bass_utils import run_bass_kernel_spmd

T = 4096
D = 1024
H = 16
DH = 64
NC = 8
HL = H // NC  # heads per core (2)
DL = HL * DH  # local feature width (128)

F32 = mybir.dt.float32
F32R = mybir.dt.float32r
BF16 = mybir.dt.bfloat16


def build_nc():
    nc = bacc.Bacc(
        "TRN2", target_bir_lowering=False, debug=False, num_devices=NC
    )

    # ---- DRAM I/O -------------------------------------------------------
    xT_d = nc.dram_tensor("xT", [D, T], BF16, kind="ExternalInput").ap()
    wqkvT_d = nc.dram_tensor("wqkvT", [D, 3 * DL], BF16, kind="ExternalInput").ap()
    woutT_d = nc.dram_tensor("woutT", [DL, D], BF16, kind="ExternalInput").ap()
    cos2_d = nc.dram_tensor("cos2", [DL, T], BF16, kind="ExternalInput").ap()
    sin2_d = nc.dram_tensor("sin2", [DL, T], BF16, kind="ExternalInput").ap()
    p128_d = nc.dram_tensor("p128", [DL, DL], BF16, kind="ExternalInput").ap()
    ident_d = nc.dram_tensor("ident", [128, 128], BF16, kind="ExternalInput").ap()
    ee_d = nc.dram_tensor("ee", [128, 2 * DL], F32R, kind="ExternalInput").ap()
    # 4 diagonal-block masks [128, 512]: mask_j[k, q] = 1 iff q >= j*128 + k
    dmask_d = nc.dram_tensor("dmask", [128, 4 * 512], BF16, kind="ExternalInput").ap()
    out_d = nc.dram_tensor("outp", [T, D], BF16, kind="ExternalOutput").ap()

    NCH = 8  # T-chunks of 512 for the QKV projection
    CW = 512  # chunk width
    VBLK = 130  # v-nat block layout: [v_h0(64) | 1 | v_h1(64) | 1]
    RING = 4  # score-ring slots (PSUM banks)

    with tile.TileContext(nc) as tc:
        with tc.tile_pool(name="consts", bufs=1) as cpool, \
             tc.tile_pool(name="persist", bufs=1) as ppool:
            # ---- constants needed immediately (QKV weights) ------------
            wt = []
            weng = [nc.sync, nc.gpsimd, nc.scalar]
            for d in range(8):
                w = cpool.tile([128, 3 * DL], BF16, tag=f"wt{d}")
                weng[d % 3].dma_start(out=w[:], in_=wqkvT_d[d * 128:(d + 1) * 128, :])
                wt.append(w)
            p128 = cpool.tile([DL, DL], BF16, tag="p128")
            nc.sync.dma_start(out=p128[:], in_=p128_d)
            ident = cpool.tile([128, 128], BF16, tag="ident")
            nc.gpsimd.dma_start(out=ident[:], in_=ident_d)

            # ---- persistent activations --------------------------------
            qT = ppool.tile([DL, T], BF16, tag="qT")
            kT = ppool.tile([DL, T], BF16, tag="kT")
            vnat = ppool.tile([128, (T // 128) * VBLK], BF16, tag="vnat")
            attnT = ppool.tile([DL, T], BF16, tag="attnT")

            # ones columns of the v-nat layout (cols 64 and 129 of each block)
            ones_sb = cpool.tile([128, T // 128], F32, tag="ones_sb")
            nc.gpsimd.memset(ones_sb[:], 1.0)
            vone = vnat[:].rearrange("p (b c) -> p b c", c=VBLK)
            nc.vector.tensor_copy(vone[:, :, 64:65], ones_sb[:].rearrange("p (b c) -> p b c", c=1))
            nc.vector.tensor_copy(vone[:, :, 129:130], ones_sb[:].rearrange("p (b c) -> p b c", c=1))

            # ---- remaining constants (deferred so x/W DMAs go first) ---
            cos2 = cpool.tile([DL, T], BF16, tag="cos2")
            sin2 = cpool.tile([DL, T], BF16, tag="sin2")
            woutT = cpool.tile([DL, D], BF16, tag="woutT")
            ee = cpool.tile([128, 2 * DL], F32R, tag="ee")
            dmask = cpool.tile([128, 4 * 512], BF16, tag="dmask")

            # ================= Phase A: QKV + RoPE ======================
            with tc.tile_pool(name="xp", bufs=2) as xpool, \
                 tc.tile_pool(name="tmpa", bufs=3) as tpool, \
                 tc.tile_pool(name="psA", bufs=2, space="PSUM") as psA:
                xeng = [nc.sync, nc.scalar, nc.gpsimd]
                for c in range(NCH):
                    s = c * CW
                    xt = xpool.tile([128, 8 * CW], BF16, tag="xchunk")
                    for d in range(8):
                        xeng[d % 3].dma_start(
                            out=xt[:, d * CW:(d + 1) * CW],
                            in_=xT_d[d * 128:(d + 1) * 128, s:s + CW],
                        )
                    if c == 0:
                        # deferred constant loads, behind the first x chunk
                        nc.gpsimd.dma_start(out=cos2[:], in_=cos2_d)
                        nc.gpsimd.dma_start(out=sin2[:], in_=sin2_d)
                        nc.sync.dma_start(out=dmask[:], in_=dmask_d)
                        nc.sync.dma_start(out=ee[:], in_=ee_d)
                        nc.scalar.dma_start(out=woutT[:], in_=woutT_d)

                    def xs(d):
                        return xt[:, d * CW:(d + 1) * CW]

                    # qT / kT with RoPE
                    for idx, dst in ((0, qT), (1, kT)):
                        pp = psA.tile([128, CW], F32, tag="qkvps")
                        for d in range(8):
                            nc.tensor.matmul(
                                pp[:],
                                lhsT=wt[d][:, idx * DL:(idx + 1) * DL],
                                rhs=xs(d),
                                start=(d == 0),
                                stop=(d == 7),
                            )
                        praw = tpool.tile([128, CW], BF16, tag="praw")
                        nc.vector.tensor_copy(praw[:], pp[:])
                        rot = psA.tile([128, CW], F32, tag="rotps")
                        nc.tensor.matmul(
                            rot[:], lhsT=p128[:], rhs=praw[:],
                            start=True, stop=True,
                        )
                        dstv = dst[:, s:s + CW]
                        nc.vector.tensor_mul(dstv, praw[:], cos2[:, s:s + CW])
                        rtmp = tpool.tile([128, CW], BF16, tag="rtmp")
                        nc.vector.tensor_mul(rtmp[:], rot[:], sin2[:, s:s + CW])
                        nc.gpsimd.tensor_add(dstv, dstv, rtmp[:])

                    # v: compute vT then PE-transpose to natural layout
                    vp = psA.tile([128, CW], F32, tag="qkvps")
                    for d in range(8):
                        nc.tensor.matmul(
                            vp[:], lhsT=wt[d][:, 2 * DL:3 * DL],
                            rhs=xs(d), start=(d == 0), stop=(d == 7),
                        )
                    vtmp = tpool.tile([128, CW], BF16, tag="vtmp")
                    nc.scalar.copy(vtmp[:], vp[:])
                    for b in range(CW // 128):
                        kb = (s // 128) + b
                        tp = psA.tile([128, 128], BF16, tag="vtps")
                        nc.tensor.transpose(
                            tp[:], vtmp[:, b * 128:(b + 1) * 128], ident[:]
                        )
                        o = kb * VBLK
                        nc.vector.tensor_copy(vnat[:, o:o + 64], tp[:, 0:64])
                        nc.vector.tensor_copy(vnat[:, o + 65:o + 129], tp[:, 64:128])

            # ====== Phase B+C: attention, normalize, out-projection =====
            # Software pipeline per q-chunk: S matmuls write a 4-slot PSUM
            # ring; exp (scalar) + diag-mask (vector) turn each slot into a
            # bf16 P tile; the AV matmuls trail the S matmuls by 4 units so
            # the PE never waits on the scalar engine.  The out-projection
            # of the PREVIOUS chunk fills the AV drain window.
            with tc.tile_pool(name="ptp", bufs=6) as ptpool, \
                 tc.tile_pool(name="evp", bufs=2) as evpool, \
                 tc.tile_pool(name="nrm", bufs=2) as npool, \
                 tc.tile_pool(name="op", bufs=3) as opool, \
                 tc.tile_pool(name="psAT", bufs=1, space="PSUM") as psAT, \
                 tc.tile_pool(name="psST", bufs=1, space="PSUM") as psST, \
                 tc.tile_pool(name="psO", bufs=2, space="PSUM") as psO:
                # persistent score ring: 4 banks; slot 2 doubles as the
                # rb broadcast target during the drain of the next chunk.
                sring = psAT.tile([128, RING * 512], F32, tag="sring", name="sring")
                oeng = [nc.sync, nc.gpsimd]

                prev = None  # (q0, rcp) of the previous chunk

                def emit_outp(qp0, rcp, part):
                    # out-projection of the previous chunk, split in parts
                    # so AV drains can interleave.
                    if part == 0:
                        rb = sring[:, 2 * 512:3 * 512]
                        for h in range(HL):
                            nc.tensor.matmul(
                                rb,
                                lhsT=ee[64:65, h * DL:(h + 1) * DL],
                                rhs=rcp[64:65, h * 512:(h + 1) * 512],
                                start=(h == 0), stop=(h == HL - 1),
                                skip_group_check=True,
                            )
                        nc.vector.tensor_mul(
                            attnT[:, qp0:qp0 + 512], attnT[:, qp0:qp0 + 512],
                            rb,
                        )
                        return
                    tbs = (0, 1) if part == 1 else (2, 3)
                    for tbl in tbs:
                        tb = qp0 // 128 + tbl
                        osb = opool.tile([128, D], BF16, tag="osb")
                        for ec in range(2):
                            op = psO.tile([128, 512], F32, tag="ops")
                            nc.tensor.matmul(
                                op[:],
                                lhsT=attnT[:, tb * 128:(tb + 1) * 128],
                                rhs=woutT[:, ec * 512:(ec + 1) * 512],
                                start=True, stop=True,
                                skip_group_check=True,
                            )
                            nc.vector.tensor_copy(
                                osb[:, ec * 512:(ec + 1) * 512], op[:])
                        oeng[tbl % 2].dma_start(
                            out=out_d[tb * 128:(tb + 1) * 128, :], in_=osb[:]
                        )

                for qc in range(9):
                    if qc < 8:
                        q0 = qc * 512
                        kmax = 4 * (qc + 1)
                        ats = [psST.tile([DH + 1, 512], F32, tag=f"atps{h}",
                                         name=f"at{h}")
                               for h in range(HL)]
                        units = [(kb, h) for kb in range(kmax)
                                 for h in range(HL)]
                        pend = []
                        outp_parts = [0, 1, 2] if qc > 0 else []

                        def flush_av():
                            kb, h, pt = pend.pop(0)
                            o = kb * VBLK + h * 65
                            nc.tensor.matmul(
                                ats[h][:],
                                lhsT=vnat[:, o:o + 65],
                                rhs=pt[:],
                                start=(kb == 0), stop=(kb == kmax - 1),
                                skip_group_check=True,
                            )

                        for ui, (kb, h) in enumerate(units):
                            slot = sring[:, (ui % RING) * 512:
                                         (ui % RING) * 512 + 512]
                            hs = h * DH
                            nc.tensor.matmul(
                                slot,
                                lhsT=kT[hs:hs + DH, kb * 128:(kb + 1) * 128],
                                rhs=qT[hs:hs + DH, q0:q0 + 512],
                                start=True, stop=True,
                                skip_group_check=True,
                            )
                            pt = ptpool.tile([128, 512], BF16, tag="pt")
                            nc.scalar.activation(
                                pt[:], slot,
                                mybir.ActivationFunctionType.Exp,
                                scale=0.125,
                            )
                            j = kb - 4 * qc
                            if j >= 0:
                                nc.vector.tensor_mul(
                                    pt[:], pt[:],
                                    dmask[:, j * 512:(j + 1) * 512],
                                )
                            pend.append((kb, h, pt))
                            if ui >= RING:
                                flush_av()

                        # drain: interleave previous chunk's out-projection
                        # (AV first — the rb matmul waits on a late exp)
                        while pend or outp_parts:
                            if pend:
                                flush_av()
                            if outp_parts:
                                emit_outp(prev[0], prev[1],
                                          outp_parts.pop(0))

                        # evacuate attn rows; reciprocal of the sums row
                        # stays in partition 64 (no cross-partition moves).
                        rcp = npool.tile([DH + 1, HL * 512], F32R, tag="rcp")
                        ssum = npool.tile([DH + 1, HL * 512], F32R, tag="ssum")
                        for h in range(HL):
                            hs = h * DH
                            ev = evpool.tile([DH + 1, 512], BF16, tag=f"ev{h}")
                            nc.vector.tensor_copy(ev[:], ats[h][:])
                            nc.sync.dma_start(
                                out=attnT[hs:hs + DH, q0:q0 + 512],
                                in_=ev[0:DH, :],
                            )
                            nc.vector.tensor_copy(
                                ssum[64:65, h * 512:(h + 1) * 512],
                                ats[h][DH:DH + 1, :],
                            )
                        with nc.allow_low_precision(reason="f32r softmax recip"):
                            nc.vector.reciprocal(
                                rcp[64:65, :], ssum[64:65, :],
                            )
                        prev = (q0, rcp)
                    else:
                        # final chunk's out-projection
                        for part in (0, 1, 2):
                            emit_outp(prev[0], prev[1], part)

    nc.compile()
    return nc


def _host_constants():
    import ml_dtypes
    inv_freq = 1.0 / (10000.0 ** (np.arange(0, DH, 2, dtype=np.float64) / DH))
    t = np.arange(T, dtype=np.float64)
    freqs = np.outer(t, inv_freq)  # [T, 32]
    emb = np.concatenate([freqs, freqs], axis=-1)  # [T, 64]
    cos = np.cos(emb).astype(np.float32).T  # [64, T]
    sin = np.sin(emb).astype(np.float32).T  # [64, T]
    sinS = sin.copy()
    sinS[0:DH // 2] *= -1.0  # fold rotate_half's negation into the table
    cos2 = np.ascontiguousarray(np.tile(cos, (HL, 1)))  # [128, T]
    sin2 = np.ascontiguousarray(np.tile(sinS, (HL, 1)))

    # swap-halves permutation (per 64-row head block), symmetric
    p1 = np.zeros((DH, DH), np.float32)
    half = DH // 2
    p1[np.arange(half), np.arange(half) + half] = 1.0
    p1[np.arange(half) + half, np.arange(half)] = 1.0
    p128 = np.block([
        [p1, np.zeros((DH, DH), np.float32)],
        [np.zeros((DH, DH), np.float32), p1],
    ]).astype(np.float32)

    ident = np.eye(128, dtype=np.float32)

    # ee[:, h*128:(h+1)*128] = e_h (one-hot 64-block) broadcast to all rows
    ee = np.zeros((128, 2 * DL), np.float32)
    for h in range(HL):
        ee[:, h * DL + h * DH:h * DL + (h + 1) * DH] = 1.0

    # diag masks [128, 4*512]: mask_j[k, q] = 1 iff q >= j*128 + k
    dmask = np.zeros((128, 4, 512), np.float32)
    kk = np.arange(128)[:, None]
    qq = np.arange(512)[None, :]
    for j in range(4):
        dmask[:, j, :] = (qq >= j * 128 + kk).astype(np.float32)
    dmask = np.ascontiguousarray(dmask.reshape(128, 4 * 512))

    bf = ml_dtypes.bfloat16
    return (cos2.astype(bf), sin2.astype(bf), p128.astype(bf),
            ident.astype(bf), ee, dmask.astype(bf))


_NC_CACHE = None


def _get_nc():
    global _NC_CACHE
    if _NC_CACHE is None:
        _NC_CACHE = build_nc()
    return _NC_CACHE


def _in_maps(x, W_qkv, W_out):
    import ml_dtypes
    bf = ml_dtypes.bfloat16
    x2 = np.asarray(x, np.float32).reshape(T, D)
    W_qkv = np.asarray(W_qkv, np.float32)
    W_out = np.asarray(W_out, np.float32)
    xT = np.ascontiguousarray(x2.T).astype(bf)
    cos2, sin2, p128, ident, ee, dmask = _host_constants()

    Wq, Wk, Wv = W_qkv[0:D], W_qkv[D:2 * D], W_qkv[2 * D:3 * D]
    in_maps = []
    for c in range(NC):
        h0, h1 = HL * c, HL * c + 1
        rows = []
        for Wp in (Wq, Wk, Wv):
            rows.append(Wp[h0 * DH:(h0 + 1) * DH])
            rows.append(Wp[h1 * DH:(h1 + 1) * DH])
        wqkvT = np.ascontiguousarray(
            np.concatenate(rows, axis=0).T).astype(bf)
        cols = np.r_[h0 * DH:(h0 + 1) * DH, h1 * DH:(h1 + 1) * DH]
        woutT = np.ascontiguousarray(W_out[:, cols].T).astype(bf)  # [128, D]
        in_maps.append({
            "xT": xT, "wqkvT": wqkvT, "woutT": woutT,
            "cos2": cos2, "sin2": sin2, "p128": p128,
            "ident": ident, "ee": ee, "dmask": dmask,
        })
    return in_maps


def _run(x, W_qkv, W_out, **spmd_kwargs):
    nc = _get_nc()
    res = run_bass_kernel_spmd(
        nc, _in_maps(x, W_qkv, W_out), core_ids=list(range(NC)), **spmd_kwargs
    )
    out = res.results[0]["outp"].astype(np.float64)
    for c in range(1, NC):
        out += res.results[c]["outp"].astype(np.float64)
    return out.astype(np.float32).reshape(1, T, D), res


def kernel(x, W_qkv, W_out):
    out, _ = _run(x, W_qkv, W_out)
    return out


# revision 12
# speedup vs baseline: 1.2959x; 1.2959x over previous
"""Causal self-attention (B=1, T=4096, D=1024, H=16, dh=64) on 8 trn2 NeuronCores.

Sharding: tensor-parallel over heads — each core owns 2 of the 16 heads.
Per core: QKV projection (transposed activation layout), RoPE, causal
flash-style attention with transposed score tiles St[k,q] (so the AV matmul
needs no P transposes), softmax denominator via an appended ones-column in V,
out-projection against this core's W_out column slice -> partial output
[T, D] in bf16.  Host sums the 8 partials in f32.

v3 (from the v1 structure, which empirically avoids sustained PE power
throttling thanks to its per-group exp stalls):
- all matmul operands bf16 (fp32r streamed at ~half PE rate on hw),
- diagonal score blocks compute only the live (unmasked) q-columns
  (-8% S/AV PE work, -12% exp work, less PE power),
- bf16 output partials (halves the output DMA; host sums in f32),
- bf16 softmax-sum bounce (half the scratch DMA),
- input/const DMAs spread across the sync/scalar/gpsimd queues.
"""

import sys

sys.path.insert(0, "/opt/trn_rl_repo")

import numpy as np

import concourse.bass as bass
import concourse.tile as tile
from concourse import bacc, mybir
from concourse.bass_utils import run_bass_kernel_spmd

T = 4096
D = 1024
H = 16
DH = 64
NC = 8
HL = H // NC  # heads per core (2)
DL = HL * DH  # local feature width (128)

F32 = mybir.dt.float32
BF16 = mybir.dt.bfloat16


def build_nc():
    nc = bacc.Bacc(
        "TRN2", target_bir_lowering=False, debug=False, num_devices=NC
    )

    # ---- DRAM I/O -------------------------------------------------------
    xT_d = nc.dram_tensor("xT", [D, T], BF16, kind="ExternalInput").ap()
    wqkvT_d = nc.dram_tensor("wqkvT", [D, 3 * DL], BF16, kind="ExternalInput").ap()
    woutT_d = nc.dram_tensor("woutT", [DL, D], BF16, kind="ExternalInput").ap()
    cos2_d = nc.dram_tensor("cos2", [DL, T], BF16, kind="ExternalInput").ap()
    sin2_d = nc.dram_tensor("sin2", [DL, T], BF16, kind="ExternalInput").ap()
    p128_d = nc.dram_tensor("p128", [DL, DL], BF16, kind="ExternalInput").ap()
    ident_d = nc.dram_tensor("ident", [128, 128], BF16, kind="ExternalInput").ap()
    e2_d = nc.dram_tensor("e2", [HL, 128], BF16, kind="ExternalInput").ap()
    # 4 diagonal-block masks [128, 512]: mask_j[k, q] = 1 iff q >= j*128 + k
    dmask_d = nc.dram_tensor("dmask", [128, 4 * 512], BF16, kind="ExternalInput").ap()
    out_d = nc.dram_tensor("outp", [T, D], BF16, kind="ExternalOutput").ap()

    # internal DRAM scratch for the softmax-sum partition shuffle
    sums_d = nc.dram_tensor("sums_scratch", [HL, T], BF16).ap()
    recip_d = nc.dram_tensor("recip_scratch", [HL, T], BF16).ap()

    NCH = 8  # T-chunks of 512 for the QKV projection
    CW = 512  # chunk width
    VBLK = 130  # v-nat block layout: [v_h0(64) | 1 | v_h1(64) | 1]

    with tile.TileContext(nc) as tc:
        with tc.tile_pool(name="consts", bufs=1) as cpool, \
             tc.tile_pool(name="persist", bufs=1) as ppool:
            # ---- constants needed immediately (QKV weights) ------------
            wt = []
            weng = [nc.sync, nc.gpsimd, nc.scalar]
            for d in range(8):
                w = cpool.tile([128, 3 * DL], BF16, tag=f"wt{d}")
                weng[d % 3].dma_start(out=w[:], in_=wqkvT_d[d * 128:(d + 1) * 128, :])
                wt.append(w)
            p128 = cpool.tile([DL, DL], BF16, tag="p128")
            nc.sync.dma_start(out=p128[:], in_=p128_d)
            ident = cpool.tile([128, 128], BF16, tag="ident")
            nc.gpsimd.dma_start(out=ident[:], in_=ident_d)

            # ---- persistent activations --------------------------------
            qT = ppool.tile([DL, T], BF16, tag="qT")
            kT = ppool.tile([DL, T], BF16, tag="kT")
            vnat = ppool.tile([128, (T // 128) * VBLK], BF16, tag="vnat")
            attnT = ppool.tile([DL, T], BF16, tag="attnT")

            # ones columns of the v-nat layout (cols 64 and 129 of each block)
            ones_sb = cpool.tile([128, T // 128], F32, tag="ones_sb")
            nc.gpsimd.memset(ones_sb[:], 1.0)
            vone = vnat[:].rearrange("p (b c) -> p b c", c=VBLK)
            nc.vector.tensor_copy(vone[:, :, 64:65], ones_sb[:].rearrange("p (b c) -> p b c", c=1))
            nc.vector.tensor_copy(vone[:, :, 129:130], ones_sb[:].rearrange("p (b c) -> p b c", c=1))

            # ---- remaining constants (deferred so x/W DMAs go first) ---
            cos2 = cpool.tile([DL, T], BF16, tag="cos2")
            sin2 = cpool.tile([DL, T], BF16, tag="sin2")
            woutT = cpool.tile([DL, D], BF16, tag="woutT")
            e2 = cpool.tile([HL, 128], BF16, tag="e2")
            dmask = cpool.tile([128, 4 * 512], BF16, tag="dmask")

            # ================= Phase A: QKV + RoPE ======================
            with tc.tile_pool(name="xp", bufs=2) as xpool, \
                 tc.tile_pool(name="tmpa", bufs=3) as tpool, \
                 tc.tile_pool(name="psA", bufs=2, space="PSUM") as psA:
                xeng = [nc.sync, nc.scalar, nc.gpsimd]
                for c in range(NCH):
                    s = c * CW
                    xt = xpool.tile([128, 8 * CW], BF16, tag="xchunk")
                    for d in range(8):
                        xeng[d % 3].dma_start(
                            out=xt[:, d * CW:(d + 1) * CW],
                            in_=xT_d[d * 128:(d + 1) * 128, s:s + CW],
                        )
                    if c == 0:
                        # deferred constant loads, behind the first x chunk
                        nc.gpsimd.dma_start(out=cos2[:], in_=cos2_d)
                        nc.gpsimd.dma_start(out=sin2[:], in_=sin2_d)
                        nc.sync.dma_start(out=dmask[:], in_=dmask_d)
                        nc.sync.dma_start(out=e2[:], in_=e2_d)
                        nc.scalar.dma_start(out=woutT[:], in_=woutT_d)

                    def xs(d):
                        return xt[:, d * CW:(d + 1) * CW]

                    # qT / kT with RoPE
                    for idx, dst in ((0, qT), (1, kT)):
                        pp = psA.tile([128, CW], F32, tag="qkvps")
                        for d in range(8):
                            nc.tensor.matmul(
                                pp[:],
                                lhsT=wt[d][:, idx * DL:(idx + 1) * DL],
                                rhs=xs(d),
                                start=(d == 0),
                                stop=(d == 7),
                            )
                        praw = tpool.tile([128, CW], BF16, tag="praw")
                        nc.vector.tensor_copy(praw[:], pp[:])
                        rot = psA.tile([128, CW], F32, tag="rotps")
                        nc.tensor.matmul(
                            rot[:], lhsT=p128[:], rhs=praw[:],
                            start=True, stop=True,
                        )
                        dstv = dst[:, s:s + CW]
                        nc.vector.tensor_mul(dstv, praw[:], cos2[:, s:s + CW])
                        rtmp = tpool.tile([128, CW], BF16, tag="rtmp")
                        nc.vector.tensor_mul(rtmp[:], rot[:], sin2[:, s:s + CW])
                        nc.gpsimd.tensor_add(dstv, dstv, rtmp[:])

                    # v: compute vT then PE-transpose to natural layout
                    vp = psA.tile([128, CW], F32, tag="qkvps")
                    for d in range(8):
                        nc.tensor.matmul(
                            vp[:], lhsT=wt[d][:, 2 * DL:3 * DL],
                            rhs=xs(d), start=(d == 0), stop=(d == 7),
                        )
                    vtmp = tpool.tile([128, CW], BF16, tag="vtmp")
                    nc.scalar.copy(vtmp[:], vp[:])
                    for b in range(CW // 128):
                        kb = (s // 128) + b
                        tp = psA.tile([128, 128], BF16, tag="vtps")
                        nc.tensor.transpose(
                            tp[:], vtmp[:, b * 128:(b + 1) * 128], ident[:]
                        )
                        o = kb * VBLK
                        nc.vector.tensor_copy(vnat[:, o:o + 64], tp[:, 0:64])
                        nc.vector.tensor_copy(vnat[:, o + 65:o + 129], tp[:, 64:128])

            # ====== Phase B+C: attention, normalize, out-projection =====
            # per q-chunk: both heads interleaved (their K=64 S-matmuls pack
            # into disjoint PE row groups), then normalize + project the
            # chunk's rows.  Diagonal blocks only compute live q-columns.
            with tc.tile_pool(name="ptp", bufs=2) as ptpool, \
                 tc.tile_pool(name="evp", bufs=3) as evpool, \
                 tc.tile_pool(name="nrm", bufs=2) as npool, \
                 tc.tile_pool(name="op", bufs=3) as opool, \
                 tc.tile_pool(name="psAT", bufs=1, space="PSUM") as psAT, \
                 tc.tile_pool(name="psST", bufs=1, space="PSUM") as psST, \
                 tc.tile_pool(name="psO", bufs=1, space="PSUM") as psO:
                rrow_prev = None
                oeng = [nc.sync, nc.gpsimd]
                for qc in range(9):
                  if qc < 8:
                    q0 = qc * 512
                    kmax = 4 * (qc + 1)
                    ats = [psAT.tile([DH + 1, 512], F32, tag=f"atps{h}", name=f"at{h}")
                           for h in range(HL)]
                    for g in range((kmax + 1) // 2):
                        kbs = [kb for kb in (2 * g, 2 * g + 1) if kb < kmax]
                        sts = [psST.tile([128, 1024], F32, tag=f"stps{h}", name=f"st{h}")
                               for h in range(HL)]
                        # live column start within the chunk, per kb (0 for
                        # off-diagonal blocks, j*128 for diagonal block j)
                        lives = [max(0, (kb - 4 * qc) * 128) for kb in kbs]
                        # S matmuls: interleave heads so the two K=64
                        # matmuls occupy PE row groups 0-63 / 64-127.
                        for i, kb in enumerate(kbs):
                            lv = lives[i]
                            for h in range(HL):
                                hs = h * DH
                                nc.tensor.matmul(
                                    sts[h][:, i * 512 + lv:(i + 1) * 512],
                                    lhsT=kT[hs:hs + DH, kb * 128:(kb + 1) * 128],
                                    rhs=qT[hs:hs + DH, q0 + lv:q0 + 512],
                                    start=True, stop=True,
                                )
                        pts = []
                        for h in range(HL):
                            pt = ptpool.tile([128, 1024], BF16, tag=f"pt{h}")
                            for i, kb in enumerate(kbs):
                                # exp only the live cols (dead cols are
                                # zeroed by the full-width mask mul below;
                                # qc==0 stays full so pt bufs initialize)
                                lv = lives[i] if qc > 0 else 0
                                nc.scalar.activation(
                                    pt[:, i * 512 + lv:(i + 1) * 512],
                                    sts[h][:, i * 512 + lv:(i + 1) * 512],
                                    mybir.ActivationFunctionType.Exp,
                                    scale=0.125,
                                )
                            pts.append(pt)
                        for i, kb in enumerate(kbs):
                            j = kb - 4 * qc
                            if j >= 0:
                                for h in range(HL):
                                    nc.vector.tensor_mul(
                                        pts[h][:, i * 512:(i + 1) * 512],
                                        pts[h][:, i * 512:(i + 1) * 512],
                                        dmask[:, j * 512:(j + 1) * 512],
                                    )
                        for i, kb in enumerate(kbs):
                            for h in range(HL):
                                o = kb * VBLK + h * 65
                                nc.tensor.matmul(
                                    ats[h][:],
                                    lhsT=vnat[:, o:o + 65],
                                    rhs=pts[h][:, i * 512:(i + 1) * 512],
                                    start=(kb == 0), stop=(kb == kmax - 1),
                                    skip_group_check=True,
                                )
                    # evacuate attn rows + sums; bounce sums through DRAM to
                    # reshape [1, 512] -> [128, 4] for a lane-parallel recip.
                    rrow = npool.tile([HL, 512], BF16, tag="rrow")
                    scom = npool.tile([128, HL * 4], BF16, tag="scom")
                    scomf = npool.tile([128, HL * 4], F32, tag="scomf")
                    rcomf = npool.tile([128, HL * 4], F32, tag="rcomf")
                    rcom = npool.tile([128, HL * 4], BF16, tag="rcom")
                    for h in range(HL):
                        hs = h * DH
                        ev = evpool.tile([DH + 1, 512], BF16, tag=f"ev{h}")
                        nc.vector.tensor_copy(ev[:], ats[h][:])
                        nc.sync.dma_start(
                            out=attnT[hs:hs + DH, q0:q0 + 512], in_=ev[0:DH, :]
                        )
                        nc.sync.dma_start(
                            out=sums_d[h, q0:q0 + 512], in_=ev[DH:DH + 1, :]
                        )
                        nc.sync.dma_start(
                            out=scom[:, h * 4:(h + 1) * 4],
                            in_=sums_d[h, q0:q0 + 512].rearrange(
                                "(p f) -> p f", p=128),
                        )
                    nc.vector.tensor_copy(scomf[:], scom[:])
                    nc.vector.reciprocal(rcomf[:], scomf[:])
                    nc.vector.tensor_copy(rcom[:], rcomf[:])
                    for h in range(HL):
                        nc.sync.dma_start(
                            out=recip_d[h, q0:q0 + 512].rearrange(
                                "(p f) -> p f", p=128),
                            in_=rcom[:, h * 4:(h + 1) * 4],
                        )
                    nc.sync.dma_start(out=rrow[:], in_=recip_d[:, q0:q0 + 512])
                    rrow_prev = rrow
                  if qc > 0:
                    # normalize + project the PREVIOUS chunk (its recip DMA
                    # chain completed under this chunk's matmuls, so the
                    # in-order PE queue never stalls on it).
                    qp0 = (qc - 1) * 512
                    rb = psO.tile([128, 512], F32, tag="rbps")
                    nc.tensor.matmul(
                        rb[:], lhsT=e2[:], rhs=rrow_p[:],
                        start=True, stop=True,
                    )
                    nc.vector.tensor_mul(
                        attnT[:, qp0:qp0 + 512], attnT[:, qp0:qp0 + 512], rb[:],
                    )
                    for tbl in range(4):
                        tb = (qc - 1) * 4 + tbl
                        osb = opool.tile([128, D], BF16, tag="osb")
                        for ec in range(2):
                            op = psO.tile([128, 512], F32, tag="ops")
                            nc.tensor.matmul(
                                op[:],
                                lhsT=attnT[:, tb * 128:(tb + 1) * 128],
                                rhs=woutT[:, ec * 512:(ec + 1) * 512],
                                start=True, stop=True,
                            )
                            nc.vector.tensor_copy(
                                osb[:, ec * 512:(ec + 1) * 512], op[:])
                        oeng[tbl % 2].dma_start(
                            out=out_d[tb * 128:(tb + 1) * 128, :], in_=osb[:]
                        )
                  rrow_p = rrow_prev

    nc.compile()
    return nc


def _host_constants():
    import ml_dtypes
    bf = ml_dtypes.bfloat16
    inv_freq = 1.0 / (10000.0 ** (np.arange(0, DH, 2, dtype=np.float64) / DH))
    t = np.arange(T, dtype=np.float64)
    freqs = np.outer(t, inv_freq)  # [T, 32]
    emb = np.concatenate([freqs, freqs], axis=-1)  # [T, 64]
    cos = np.cos(emb).astype(np.float32).T  # [64, T]
    sin = np.sin(emb).astype(np.float32).T  # [64, T]
    sinS = sin.copy()
    sinS[0:DH // 2] *= -1.0  # fold rotate_half's negation into the table
    cos2 = np.ascontiguousarray(np.tile(cos, (HL, 1)))  # [128, T]
    sin2 = np.ascontiguousarray(np.tile(sinS, (HL, 1)))

    # swap-halves permutation (per 64-row head block), symmetric
    p1 = np.zeros((DH, DH), np.float32)
    half = DH // 2
    p1[np.arange(half), np.arange(half) + half] = 1.0
    p1[np.arange(half) + half, np.arange(half)] = 1.0
    p128 = np.block([
        [p1, np.zeros((DH, DH), np.float32)],
        [np.zeros((DH, DH), np.float32), p1],
    ]).astype(np.float32)

    ident = np.eye(128, dtype=np.float32)

    e2 = np.zeros((HL, 128), np.float32)
    for h in range(HL):
        e2[h, h * DH:(h + 1) * DH] = 1.0

    # diag masks [128, 4*512]: mask_j[k, q] = 1 iff q >= j*128 + k
    dmask = np.zeros((128, 4, 512), np.float32)
    kk = np.arange(128)[:, None]
    qq = np.arange(512)[None, :]
    for j in range(4):
        dmask[:, j, :] = (qq >= j * 128 + kk).astype(np.float32)
    dmask = np.ascontiguousarray(dmask.reshape(128, 4 * 512))

    return (cos2.astype(bf), sin2.astype(bf), p128.astype(bf),
            ident.astype(bf), e2.astype(bf), dmask.astype(bf))


_NC_CACHE = None


def _get_nc():
    global _NC_CACHE
    if _NC_CACHE is None:
        _NC_CACHE = build_nc()
    return _NC_CACHE


def _in_maps(x, W_qkv, W_out):
    import ml_dtypes
    bf = ml_dtypes.bfloat16
    x2 = np.asarray(x, np.float32).reshape(T, D)
    W_qkv = np.asarray(W_qkv, np.float32)
    W_out = np.asarray(W_out, np.float32)
    xT = np.ascontiguousarray(x2.T).astype(bf)
    cos2, sin2, p128, ident, e2, dmask = _host_constants()

    Wq, Wk, Wv = W_qkv[0:D], W_qkv[D:2 * D], W_qkv[2 * D:3 * D]
    in_maps = []
    for c in range(NC):
        h0, h1 = HL * c, HL * c + 1
        rows = []
        for Wp in (Wq, Wk, Wv):
            rows.append(Wp[h0 * DH:(h0 + 1) * DH])
            rows.append(Wp[h1 * DH:(h1 + 1) * DH])
        wqkvT = np.ascontiguousarray(
            np.concatenate(rows, axis=0).T).astype(bf)
        cols = np.r_[h0 * DH:(h0 + 1) * DH, h1 * DH:(h1 + 1) * DH]
        woutT = np.ascontiguousarray(W_out[:, cols].T).astype(bf)  # [128, D]
        in_maps.append({
            "xT": xT, "wqkvT": wqkvT, "woutT": woutT,
            "cos2": cos2, "sin2": sin2, "p128": p128,
            "ident": ident, "e2": e2, "dmask": dmask,
        })
    return in_maps


def _run(x, W_qkv, W_out, **spmd_kwargs):
    nc = _get_nc()
    res = run_bass_kernel_spmd(
        nc, _in_maps(x, W_qkv, W_out), core_ids=list(range(NC)), **spmd_kwargs
    )
    out = res.results[0]["outp"].astype(np.float64)
    for c in range(1, NC):
        out += res.results[c]["outp"].astype(np.float64)
    return out.astype(np.float32).reshape(1, T, D), res


def kernel(x, W_qkv, W_out):
    out, _ = _run(x, W_qkv, W_out)
    return out


# revision 16
# speedup vs baseline: 1.3194x; 1.0182x over previous
"""Causal self-attention (B=1, T=4096, D=1024, H=16, dh=64) on 8 trn2 NeuronCores.

Sharding: tensor-parallel over heads — each core owns 2 of the 16 heads.
Per core: QKV projection (transposed activation layout), RoPE, causal
flash-style attention with transposed score tiles St[k,q] (so the AV matmul
needs no P transposes), softmax denominator via an appended ones-column in V,
out-projection against this core's W_out column slice -> partial output
[T, D] in bf16.  Host sums the 8 partials in f32.

v3 (from the v1 structure, which empirically avoids sustained PE power
throttling thanks to its per-group exp stalls):
- all matmul operands bf16 (fp32r streamed at ~half PE rate on hw),
- diagonal score blocks compute only the live (unmasked) q-columns
  (-8% S/AV PE work, -12% exp work, less PE power),
- bf16 output partials (halves the output DMA; host sums in f32),
- bf16 softmax-sum bounce (half the scratch DMA),
- input/const DMAs spread across the sync/scalar/gpsimd queues.
"""

import sys

sys.path.insert(0, "/opt/trn_rl_repo")

import numpy as np

import concourse.bass as bass
import concourse.tile as tile
from concourse import bacc, mybir
from concourse.bass_utils import run_bass_kernel_spmd

T = 4096
D = 1024
H = 16
DH = 64
NC = 8
HL = H // NC  # heads per core (2)
DL = HL * DH  # local feature width (128)

F32 = mybir.dt.float32
BF16 = mybir.dt.bfloat16


def build_nc():
    nc = bacc.Bacc(
        "TRN2", target_bir_lowering=False, debug=False, num_devices=NC
    )

    # ---- DRAM I/O -------------------------------------------------------
    xT_d = nc.dram_tensor("xT", [D, T], BF16, kind="ExternalInput").ap()
    wqkvT_d = nc.dram_tensor("wqkvT", [D, 3 * DL], BF16, kind="ExternalInput").ap()
    woutT_d = nc.dram_tensor("woutT", [DL, D], BF16, kind="ExternalInput").ap()
    cos2_d = nc.dram_tensor("cos2", [DL, T], BF16, kind="ExternalInput").ap()
    sin2_d = nc.dram_tensor("sin2", [DL, T], BF16, kind="ExternalInput").ap()
    p128_d = nc.dram_tensor("p128", [DL, DL], BF16, kind="ExternalInput").ap()
    ident_d = nc.dram_tensor("ident", [128, 128], BF16, kind="ExternalInput").ap()
    e2_d = nc.dram_tensor("e2", [HL, 128], BF16, kind="ExternalInput").ap()
    # 4 diagonal-block masks [128, 512]: mask_j[k, q] = 1 iff q >= j*128 + k
    dmask_d = nc.dram_tensor("dmask", [128, 4 * 512], BF16, kind="ExternalInput").ap()
    out_d = nc.dram_tensor("outp", [T, D], BF16, kind="ExternalOutput").ap()

    # internal DRAM scratch for the softmax-sum partition shuffle
    sums_d = nc.dram_tensor("sums_scratch", [HL, T], F32).ap()
    recip_d = nc.dram_tensor("recip_scratch", [HL, T], BF16).ap()

    NCH = 8  # T-chunks of 512 for the QKV projection
    CW = 512  # chunk width
    VBLK = 130  # v-nat block layout: [v_h0(64) | 1 | v_h1(64) | 1]

    with tile.TileContext(nc) as tc:
        with tc.tile_pool(name="consts", bufs=1) as cpool, \
             tc.tile_pool(name="persist", bufs=1) as ppool:
            # ---- constants needed immediately (QKV weights) ------------
            wt = []
            weng = [nc.sync, nc.gpsimd, nc.scalar]
            for d in range(8):
                w = cpool.tile([128, 3 * DL], BF16, tag=f"wt{d}")
                weng[d % 3].dma_start(out=w[:], in_=wqkvT_d[d * 128:(d + 1) * 128, :])
                wt.append(w)
            p128 = cpool.tile([DL, DL], BF16, tag="p128")
            nc.sync.dma_start(out=p128[:], in_=p128_d)
            ident = cpool.tile([128, 128], BF16, tag="ident")
            nc.gpsimd.dma_start(out=ident[:], in_=ident_d)

            # ---- persistent activations --------------------------------
            qT = ppool.tile([DL, T], BF16, tag="qT")
            kT = ppool.tile([DL, T], BF16, tag="kT")
            vnat = ppool.tile([128, (T // 128) * VBLK], BF16, tag="vnat")
            attnT = ppool.tile([DL, T], BF16, tag="attnT")

            # ones columns of the v-nat layout (cols 64 and 129 of each block)
            ones_sb = cpool.tile([128, T // 128], F32, tag="ones_sb")
            nc.gpsimd.memset(ones_sb[:], 1.0)
            vone = vnat[:].rearrange("p (b c) -> p b c", c=VBLK)
            nc.vector.tensor_copy(vone[:, :, 64:65], ones_sb[:].rearrange("p (b c) -> p b c", c=1))
            nc.vector.tensor_copy(vone[:, :, 129:130], ones_sb[:].rearrange("p (b c) -> p b c", c=1))

            # ---- remaining constants (deferred so x/W DMAs go first) ---
            cos2 = cpool.tile([DL, T], BF16, tag="cos2")
            sin2 = cpool.tile([DL, T], BF16, tag="sin2")
            woutT = cpool.tile([DL, D], BF16, tag="woutT")
            e2 = cpool.tile([HL, 128], BF16, tag="e2")
            dmask = cpool.tile([128, 4 * 512], BF16, tag="dmask")

            # ================= Phase A: QKV + RoPE ======================
            with tc.tile_pool(name="xp", bufs=2) as xpool, \
                 tc.tile_pool(name="tmpa", bufs=3) as tpool, \
                 tc.tile_pool(name="psA", bufs=2, space="PSUM") as psA:
                xeng = [nc.sync, nc.scalar, nc.gpsimd]
                for c in range(NCH):
                    s = c * CW
                    xt = xpool.tile([128, 8 * CW], BF16, tag="xchunk")
                    for d in range(8):
                        xeng[d % 3].dma_start(
                            out=xt[:, d * CW:(d + 1) * CW],
                            in_=xT_d[d * 128:(d + 1) * 128, s:s + CW],
                        )
                    if c == 0:
                        # deferred constant loads, behind the first x chunk
                        nc.gpsimd.dma_start(out=cos2[:], in_=cos2_d)
                        nc.gpsimd.dma_start(out=sin2[:], in_=sin2_d)
                        nc.sync.dma_start(out=dmask[:], in_=dmask_d)
                        nc.sync.dma_start(out=e2[:], in_=e2_d)
                        nc.scalar.dma_start(out=woutT[:], in_=woutT_d)

                    def xs(d):
                        return xt[:, d * CW:(d + 1) * CW]

                    # qT / kT with RoPE
                    for idx, dst in ((0, qT), (1, kT)):
                        pp = psA.tile([128, CW], F32, tag="qkvps")
                        for d in range(8):
                            nc.tensor.matmul(
                                pp[:],
                                lhsT=wt[d][:, idx * DL:(idx + 1) * DL],
                                rhs=xs(d),
                                start=(d == 0),
                                stop=(d == 7),
                            )
                        praw = tpool.tile([128, CW], BF16, tag="praw")
                        nc.vector.tensor_copy(praw[:], pp[:])
                        rot = psA.tile([128, CW], F32, tag="rotps")
                        nc.tensor.matmul(
                            rot[:], lhsT=p128[:], rhs=praw[:],
                            start=True, stop=True,
                        )
                        dstv = dst[:, s:s + CW]
                        nc.vector.tensor_mul(dstv, praw[:], cos2[:, s:s + CW])
                        rtmp = tpool.tile([128, CW], BF16, tag="rtmp")
                        nc.vector.tensor_mul(rtmp[:], rot[:], sin2[:, s:s + CW])
                        nc.gpsimd.tensor_add(dstv, dstv, rtmp[:])

                    # v: compute vT then PE-transpose to natural layout
                    vp = psA.tile([128, CW], F32, tag="qkvps")
                    for d in range(8):
                        nc.tensor.matmul(
                            vp[:], lhsT=wt[d][:, 2 * DL:3 * DL],
                            rhs=xs(d), start=(d == 0), stop=(d == 7),
                        )
                    vtmp = tpool.tile([128, CW], BF16, tag="vtmp")
                    nc.scalar.copy(vtmp[:], vp[:])
                    for b in range(CW // 128):
                        kb = (s // 128) + b
                        tp = psA.tile([128, 128], BF16, tag="vtps")
                        nc.tensor.transpose(
                            tp[:], vtmp[:, b * 128:(b + 1) * 128], ident[:]
                        )
                        o = kb * VBLK
                        nc.vector.tensor_copy(vnat[:, o:o + 64], tp[:, 0:64])
                        nc.vector.tensor_copy(vnat[:, o + 65:o + 129], tp[:, 64:128])

            # ====== Phase B+C: attention, normalize, out-projection =====
            # per q-chunk: both heads interleaved (their K=64 S-matmuls pack
            # into disjoint PE row groups), then normalize + project the
            # chunk's rows.  Diagonal blocks only compute live q-columns.
            with tc.tile_pool(name="ptp", bufs=2) as ptpool, \
                 tc.tile_pool(name="evp", bufs=3) as evpool, \
                 tc.tile_pool(name="nrm", bufs=2) as npool, \
                 tc.tile_pool(name="op", bufs=3) as opool, \
                 tc.tile_pool(name="psAT", bufs=1, space="PSUM") as psAT, \
                 tc.tile_pool(name="psST", bufs=1, space="PSUM") as psST, \
                 tc.tile_pool(name="psO", bufs=1, space="PSUM") as psO:
                rrow_prev = None
                oeng = [nc.sync, nc.gpsimd]

                def emit_outp(qcp, rrow_q):
                    # normalize + project chunk qcp (recip chain already
                    # resolved under the next chunk's matmuls)
                    qp0 = qcp * 512
                    rb = psO.tile([128, 512], F32, tag="rbps")
                    nc.tensor.matmul(
                        rb[:], lhsT=e2[:], rhs=rrow_q[:],
                        start=True, stop=True,
                        skip_group_check=True,
                    )
                    nc.vector.tensor_mul(
                        attnT[:, qp0:qp0 + 512], attnT[:, qp0:qp0 + 512], rb[:],
                    )
                    for tbl in range(4):
                        tb = qcp * 4 + tbl
                        osb = opool.tile([128, D], BF16, tag="osb")
                        for ec in range(2):
                            op = psO.tile([128, 512], F32, tag="ops")
                            nc.tensor.matmul(
                                op[:],
                                lhsT=attnT[:, tb * 128:(tb + 1) * 128],
                                rhs=woutT[:, ec * 512:(ec + 1) * 512],
                                start=True, stop=True,
                                skip_group_check=True,
                            )
                            nc.vector.tensor_copy(
                                osb[:, ec * 512:(ec + 1) * 512], op[:])
                        oeng[tbl % 2].dma_start(
                            out=out_d[tb * 128:(tb + 1) * 128, :], in_=osb[:]
                        )

                for qc in range(9):
                  if qc < 8:
                    q0 = qc * 512
                    kmax = 4 * (qc + 1)
                    ats = [psAT.tile([DH + 1, 512], F32, tag=f"atps{h}", name=f"at{h}")
                           for h in range(HL)]
                    for g in range((kmax + 1) // 2):
                        kbs = [kb for kb in (2 * g, 2 * g + 1) if kb < kmax]
                        sts = [psST.tile([128, 1024], F32, tag=f"stps{h}", name=f"st{h}")
                               for h in range(HL)]
                        # live column start within the chunk, per kb (0 for
                        # off-diagonal blocks, j*128 for diagonal block j)
                        lives = [max(0, (kb - 4 * qc) * 128) for kb in kbs]
                        # S matmuls: interleave heads so the two K=64
                        # matmuls occupy PE row groups 0-63 / 64-127.
                        for i, kb in enumerate(kbs):
                            lv = lives[i]
                            for h in range(HL):
                                hs = h * DH
                                nc.tensor.matmul(
                                    sts[h][:, i * 512 + lv:(i + 1) * 512],
                                    lhsT=kT[hs:hs + DH, kb * 128:(kb + 1) * 128],
                                    rhs=qT[hs:hs + DH, q0 + lv:q0 + 512],
                                    start=True, stop=True,
                                )
                        pts = []
                        for h in range(HL):
                            pt = ptpool.tile([128, 1024], BF16, tag=f"pt{h}")
                            if qc > 0 and max(lives) > 0:
                                for i, kb in enumerate(kbs):
                                    # exp only the live cols (dead cols are
                                    # zeroed by the full-width mask mul)
                                    lv = lives[i]
                                    nc.scalar.activation(
                                        pt[:, i * 512 + lv:(i + 1) * 512],
                                        sts[h][:, i * 512 + lv:(i + 1) * 512],
                                        mybir.ActivationFunctionType.Exp,
                                        scale=0.125,
                                    )
                            else:
                                nc.scalar.activation(
                                    pt[:], sts[h][:],
                                    mybir.ActivationFunctionType.Exp,
                                    scale=0.125,
                                )
                            pts.append(pt)
                        for i, kb in enumerate(kbs):
                            j = kb - 4 * qc
                            if j >= 0:
                                for h in range(HL):
                                    nc.vector.tensor_mul(
                                        pts[h][:, i * 512:(i + 1) * 512],
                                        pts[h][:, i * 512:(i + 1) * 512],
                                        dmask[:, j * 512:(j + 1) * 512],
                                    )
                        for i, kb in enumerate(kbs):
                            for h in range(HL):
                                o = kb * VBLK + h * 65
                                nc.tensor.matmul(
                                    ats[h][:],
                                    lhsT=vnat[:, o:o + 65],
                                    rhs=pts[h][:, i * 512:(i + 1) * 512],
                                    start=(kb == 0), stop=(kb == kmax - 1),
                                    skip_group_check=True,
                                )
                        if g == 0 and qc > 0:
                            # previous chunk's out-projection fills the
                            # exp pipeline warm-up stall of this chunk
                            emit_outp(qc - 1, rrow_prev)
                    # evacuate attn rows + sums; reshape the sums row
                    # [1, 512] -> [128, 4] with direct SBUF rearrange DMAs
                    # for a lane-parallel reciprocal (no DRAM bounce).
                    rrow = npool.tile([HL, 512], BF16, tag="rrow")
                    scomf = npool.tile([128, HL * 4], F32, tag="scomf")
                    rcomf = npool.tile([128, HL * 4], F32, tag="rcomf")
                    rcom = npool.tile([128, HL * 4], BF16, tag="rcom")
                    for h in range(HL):
                        hs = h * DH
                        ev = evpool.tile([DH + 1, 512], BF16, tag=f"ev{h}")
                        ssr = evpool.tile([DH + 1, 512], F32, tag=f"ssr{h}")
                        nc.vector.tensor_copy(ev[:], ats[h][:])
                        nc.sync.dma_start(
                            out=attnT[hs:hs + DH, q0:q0 + 512], in_=ev[0:DH, :]
                        )
                        nc.vector.tensor_copy(
                            ssr[DH:DH + 1, :], ats[h][DH:DH + 1, :])
                        nc.gpsimd.dma_start(
                            out=sums_d[h, q0:q0 + 512], in_=ssr[DH:DH + 1, :]
                        )
                        nc.gpsimd.dma_start(
                            out=scomf[:, h * 4:(h + 1) * 4],
                            in_=sums_d[h, q0:q0 + 512].rearrange(
                                "(p f) -> p f", p=128),
                        )
                    nc.vector.reciprocal(rcomf[:], scomf[:])
                    nc.vector.tensor_copy(rcom[:], rcomf[:])
                    for h in range(HL):
                        nc.gpsimd.dma_start(
                            out=recip_d[h, q0:q0 + 512].rearrange(
                                "(p f) -> p f", p=128),
                            in_=rcom[:, h * 4:(h + 1) * 4],
                        )
                    nc.gpsimd.dma_start(
                        out=rrow[:], in_=recip_d[:, q0:q0 + 512])
                    rrow_prev = rrow
                  else:
                    # final chunk's out-projection
                    emit_outp(7, rrow_prev)

    nc.compile()
    return nc


def _host_constants():
    import ml_dtypes
    bf = ml_dtypes.bfloat16
    inv_freq = 1.0 / (10000.0 ** (np.arange(0, DH, 2, dtype=np.float64) / DH))
    t = np.arange(T, dtype=np.float64)
    freqs = np.outer(t, inv_freq)  # [T, 32]
    emb = np.concatenate([freqs, freqs], axis=-1)  # [T, 64]
    cos = np.cos(emb).astype(np.float32).T  # [64, T]
    sin = np.sin(emb).astype(np.float32).T  # [64, T]
    sinS = sin.copy()
    sinS[0:DH // 2] *= -1.0  # fold rotate_half's negation into the table
    cos2 = np.ascontiguousarray(np.tile(cos, (HL, 1)))  # [128, T]
    sin2 = np.ascontiguousarray(np.tile(sinS, (HL, 1)))

    # swap-halves permutation (per 64-row head block), symmetric
    p1 = np.zeros((DH, DH), np.float32)
    half = DH // 2
    p1[np.arange(half), np.arange(half) + half] = 1.0
    p1[np.arange(half) + half, np.arange(half)] = 1.0
    p128 = np.block([
        [p1, np.zeros((DH, DH), np.float32)],
        [np.zeros((DH, DH), np.float32), p1],
    ]).astype(np.float32)

    ident = np.eye(128, dtype=np.float32)

    e2 = np.zeros((HL, 128), np.float32)
    for h in range(HL):
        e2[h, h * DH:(h + 1) * DH] = 1.0

    # diag masks [128, 4*512]: mask_j[k, q] = 1 iff q >= j*128 + k
    dmask = np.zeros((128, 4, 512), np.float32)
    kk = np.arange(128)[:, None]
    qq = np.arange(512)[None, :]
    for j in range(4):
        dmask[:, j, :] = (qq >= j * 128 + kk).astype(np.float32)
    dmask = np.ascontiguousarray(dmask.reshape(128, 4 * 512))

    return (cos2.astype(bf), sin2.astype(bf), p128.astype(bf),
            ident.astype(bf), e2.astype(bf), dmask.astype(bf))


_NC_CACHE = None


def _get_nc():
    global _NC_CACHE
    if _NC_CACHE is None:
        _NC_CACHE = build_nc()
    return _NC_CACHE


def _in_maps(x, W_qkv, W_out):
    import ml_dtypes
    bf = ml_dtypes.bfloat16
    x2 = np.asarray(x, np.float32).reshape(T, D)
    W_qkv = np.asarray(W_qkv, np.float32)
    W_out = np.asarray(W_out, np.float32)
    xT = np.ascontiguousarray(x2.T).astype(bf)
    cos2, sin2, p128, ident, e2, dmask = _host_constants()

    Wq, Wk, Wv = W_qkv[0:D], W_qkv[D:2 * D], W_qkv[2 * D:3 * D]
    in_maps = []
    for c in range(NC):
        h0, h1 = HL * c, HL * c + 1
        rows = []
        for Wp in (Wq, Wk, Wv):
            rows.append(Wp[h0 * DH:(h0 + 1) * DH])
            rows.append(Wp[h1 * DH:(h1 + 1) * DH])
        wqkvT = np.ascontiguousarray(
            np.concatenate(rows, axis=0).T).astype(bf)
        cols = np.r_[h0 * DH:(h0 + 1) * DH, h1 * DH:(h1 + 1) * DH]
        woutT = np.ascontiguousarray(W_out[:, cols].T).astype(bf)  # [128, D]
        in_maps.append({
            "xT": xT, "wqkvT": wqkvT, "woutT": woutT,
            "cos2": cos2, "sin2": sin2, "p128": p128,
            "ident": ident, "e2": e2, "dmask": dmask,
        })
    return in_maps


def _run(x, W_qkv, W_out, **spmd_kwargs):
    nc = _get_nc()
    res = run_bass_kernel_spmd(
        nc, _in_maps(x, W_qkv, W_out), core_ids=list(range(NC)), **spmd_kwargs
    )
    out = res.results[0]["outp"].astype(np.float64)
    for c in range(1, NC):
        out += res.results[c]["outp"].astype(np.float64)
    return out.astype(np.float32).reshape(1, T, D), res


def kernel(x, W_qkv, W_out):
    out, _ = _run(x, W_qkv, W_out)
    return out


# revision 18
# speedup vs baseline: 1.3239x; 1.0033x over previous
"""Causal self-attention (B=1, T=4096, D=1024, H=16, dh=64) on 8 trn2 NeuronCores.

Sharding: tensor-parallel over heads — each core owns 2 of the 16 heads.
Per core: QKV projection (transposed activation layout), RoPE, causal
flash-style attention with transposed score tiles St[k,q] (so the AV matmul
needs no P transposes), softmax denominator via an appended ones-column in V,
out-projection against this core's W_out column slice -> partial output
[T, D] in bf16.  Host sums the 8 partials in f32.

v3 (from the v1 structure, which empirically avoids sustained PE power
throttling thanks to its per-group exp stalls):
- all matmul operands bf16 (fp32r streamed at ~half PE rate on hw),
- diagonal score blocks compute only the live (unmasked) q-columns
  (-8% S/AV PE work, -12% exp work, less PE power),
- bf16 output partials (halves the output DMA; host sums in f32),
- bf16 softmax-sum bounce (half the scratch DMA),
- input/const DMAs spread across the sync/scalar/gpsimd queues.
"""

import sys

sys.path.insert(0, "/opt/trn_rl_repo")

import numpy as np

import concourse.bass as bass
import concourse.tile as tile
from concourse import bacc, mybir
from concourse.bass_utils import run_bass_kernel_spmd

T = 4096
D = 1024
H = 16
DH = 64
NC = 8
HL = H // NC  # heads per core (2)
DL = HL * DH  # local feature width (128)

F32 = mybir.dt.float32
BF16 = mybir.dt.bfloat16


def build_nc():
    nc = bacc.Bacc(
        "TRN2", target_bir_lowering=False, debug=False, num_devices=NC
    )

    # ---- DRAM I/O -------------------------------------------------------
    xT_d = nc.dram_tensor("xT", [D, T], BF16, kind="ExternalInput").ap()
    wqkvT_d = nc.dram_tensor("wqkvT", [D, 3 * DL], BF16, kind="ExternalInput").ap()
    woutT_d = nc.dram_tensor("woutT", [DL, D], BF16, kind="ExternalInput").ap()
    cos2_d = nc.dram_tensor("cos2", [DL, T], BF16, kind="ExternalInput").ap()
    sin2_d = nc.dram_tensor("sin2", [DL, T], BF16, kind="ExternalInput").ap()
    p128_d = nc.dram_tensor("p128", [DL, DL], BF16, kind="ExternalInput").ap()
    ident_d = nc.dram_tensor("ident", [128, 128], BF16, kind="ExternalInput").ap()
    e2_d = nc.dram_tensor("e2", [HL, 128], BF16, kind="ExternalInput").ap()
    # 4 diagonal-block masks [128, 512]: mask_j[k, q] = 1 iff q >= j*128 + k
    dmask_d = nc.dram_tensor("dmask", [128, 4 * 512], BF16, kind="ExternalInput").ap()
    out_d = nc.dram_tensor("outp", [T, D], BF16, kind="ExternalOutput").ap()

    # internal DRAM scratch for the softmax-sum partition shuffle
    sums_d = nc.dram_tensor("sums_scratch", [HL, T], F32).ap()
    recip_d = nc.dram_tensor("recip_scratch", [HL, T], BF16).ap()

    NCH = 8  # T-chunks of 512 for the QKV projection
    CW = 512  # chunk width
    VBLK = 130  # v-nat block layout: [v_h0(64) | 1 | v_h1(64) | 1]

    with tile.TileContext(nc) as tc:
        with tc.tile_pool(name="consts", bufs=1) as cpool, \
             tc.tile_pool(name="persist", bufs=1) as ppool:
            # ---- constants needed immediately (QKV weights) ------------
            weng = [nc.sync, nc.gpsimd, nc.scalar]
            # first x chunk goes FIRST on every queue, interleaved with the
            # matching weight block, so QKV matmul d can start as soon as
            # (x_d, w_d) land.
            xt0 = None  # created below, before the weights
            wt = [cpool.tile([128, 3 * DL], BF16, tag=f"wt{d}", name=f"wt{d}")
                  for d in range(8)]
            p128 = cpool.tile([DL, DL], BF16, tag="p128")
            ident = cpool.tile([128, 128], BF16, tag="ident")

            # ---- persistent activations --------------------------------
            qT = ppool.tile([DL, T], BF16, tag="qT")
            kT = ppool.tile([DL, T], BF16, tag="kT")
            vnat = ppool.tile([128, (T // 128) * VBLK], BF16, tag="vnat")
            attnT = ppool.tile([DL, T], BF16, tag="attnT")

            # ones columns of the v-nat layout (cols 64 and 129 of each block)
            ones_sb = cpool.tile([128, T // 128], F32, tag="ones_sb")
            nc.gpsimd.memset(ones_sb[:], 1.0)
            vone = vnat[:].rearrange("p (b c) -> p b c", c=VBLK)
            nc.vector.tensor_copy(vone[:, :, 64:65], ones_sb[:].rearrange("p (b c) -> p b c", c=1))
            nc.vector.tensor_copy(vone[:, :, 129:130], ones_sb[:].rearrange("p (b c) -> p b c", c=1))

            # ---- remaining constants (deferred so x/W DMAs go first) ---
            cos2 = cpool.tile([DL, T], BF16, tag="cos2")
            sin2 = cpool.tile([DL, T], BF16, tag="sin2")
            woutT = cpool.tile([DL, D], BF16, tag="woutT")
            e2 = cpool.tile([HL, 128], BF16, tag="e2")
            dmask = cpool.tile([128, 4 * 512], BF16, tag="dmask")

            # ================= Phase A: QKV + RoPE ======================
            with tc.tile_pool(name="xp", bufs=3) as xpool, \
                 tc.tile_pool(name="tmpa", bufs=3) as tpool, \
                 tc.tile_pool(name="psA", bufs=2, space="PSUM") as psA:
                xeng = [nc.sync, nc.scalar, nc.gpsimd]
                for c in range(NCH):
                    s = c * CW
                    xt = xpool.tile([128, 8 * CW], BF16, tag="xchunk")
                    for d in range(8):
                        xeng[d % 3].dma_start(
                            out=xt[:, d * CW:(d + 1) * CW],
                            in_=xT_d[d * 128:(d + 1) * 128, s:s + CW],
                        )
                        if c == 0:
                            # pair each first-chunk x block with its weight
                            weng[d % 3].dma_start(
                                out=wt[d][:],
                                in_=wqkvT_d[d * 128:(d + 1) * 128, :])
                    if c == 0:
                        # deferred constant loads, behind the first x chunk
                        nc.sync.dma_start(out=p128[:], in_=p128_d)
                        nc.gpsimd.dma_start(out=ident[:], in_=ident_d)
                        nc.gpsimd.dma_start(out=cos2[:], in_=cos2_d)
                        nc.gpsimd.dma_start(out=sin2[:], in_=sin2_d)
                        nc.sync.dma_start(out=dmask[:], in_=dmask_d)
                        nc.sync.dma_start(out=e2[:], in_=e2_d)
                        nc.scalar.dma_start(out=woutT[:], in_=woutT_d)

                    def xs(d):
                        return xt[:, d * CW:(d + 1) * CW]

                    # qT / kT with RoPE
                    for idx, dst in ((0, qT), (1, kT)):
                        pp = psA.tile([128, CW], F32, tag="qkvps")
                        for d in range(8):
                            nc.tensor.matmul(
                                pp[:],
                                lhsT=wt[d][:, idx * DL:(idx + 1) * DL],
                                rhs=xs(d),
                                start=(d == 0),
                                stop=(d == 7),
                            )
                        praw = tpool.tile([128, CW], BF16, tag="praw")
                        nc.vector.tensor_copy(praw[:], pp[:])
                        rot = psA.tile([128, CW], F32, tag="rotps")
                        nc.tensor.matmul(
                            rot[:], lhsT=p128[:], rhs=praw[:],
                            start=True, stop=True,
                        )
                        dstv = dst[:, s:s + CW]
                        nc.vector.tensor_mul(dstv, praw[:], cos2[:, s:s + CW])
                        rtmp = tpool.tile([128, CW], BF16, tag="rtmp")
                        nc.vector.tensor_mul(rtmp[:], rot[:], sin2[:, s:s + CW])
                        nc.gpsimd.tensor_add(dstv, dstv, rtmp[:])

                    # v: compute vT then PE-transpose to natural layout
                    vp = psA.tile([128, CW], F32, tag="qkvps")
                    for d in range(8):
                        nc.tensor.matmul(
                            vp[:], lhsT=wt[d][:, 2 * DL:3 * DL],
                            rhs=xs(d), start=(d == 0), stop=(d == 7),
                        )
                    vtmp = tpool.tile([128, CW], BF16, tag="vtmp")
                    nc.scalar.copy(vtmp[:], vp[:])
                    for b in range(CW // 128):
                        kb = (s // 128) + b
                        tp = psA.tile([128, 128], BF16, tag="vtps")
                        nc.tensor.transpose(
                            tp[:], vtmp[:, b * 128:(b + 1) * 128], ident[:]
                        )
                        o = kb * VBLK
                        nc.vector.tensor_copy(vnat[:, o:o + 64], tp[:, 0:64])
                        nc.vector.tensor_copy(vnat[:, o + 65:o + 129], tp[:, 64:128])

            # ====== Phase B+C: attention, normalize, out-projection =====
            # per q-chunk: both heads interleaved (their K=64 S-matmuls pack
            # into disjoint PE row groups), then normalize + project the
            # chunk's rows.  Diagonal blocks only compute live q-columns.
            with tc.tile_pool(name="ptp", bufs=2) as ptpool, \
                 tc.tile_pool(name="evp", bufs=3) as evpool, \
                 tc.tile_pool(name="nrm", bufs=2) as npool, \
                 tc.tile_pool(name="op", bufs=3) as opool, \
                 tc.tile_pool(name="psAT", bufs=1, space="PSUM") as psAT, \
                 tc.tile_pool(name="psST", bufs=1, space="PSUM") as psST, \
                 tc.tile_pool(name="psO", bufs=1, space="PSUM") as psO:
                rrow_prev = None
                oeng = [nc.sync, nc.gpsimd]

                def emit_outp(qcp, rrow_q):
                    # normalize + project chunk qcp (recip chain already
                    # resolved under the next chunk's matmuls)
                    qp0 = qcp * 512
                    rb = psO.tile([128, 512], F32, tag="rbps")
                    nc.tensor.matmul(
                        rb[:], lhsT=e2[:], rhs=rrow_q[:],
                        start=True, stop=True,
                        skip_group_check=True,
                    )
                    nc.vector.tensor_mul(
                        attnT[:, qp0:qp0 + 512], attnT[:, qp0:qp0 + 512], rb[:],
                    )
                    for tbl in range(4):
                        tb = qcp * 4 + tbl
                        osb = opool.tile([128, D], BF16, tag="osb")
                        for ec in range(2):
                            op = psO.tile([128, 512], F32, tag="ops")
                            nc.tensor.matmul(
                                op[:],
                                lhsT=attnT[:, tb * 128:(tb + 1) * 128],
                                rhs=woutT[:, ec * 512:(ec + 1) * 512],
                                start=True, stop=True,
                                skip_group_check=True,
                            )
                            nc.vector.tensor_copy(
                                osb[:, ec * 512:(ec + 1) * 512], op[:])
                        oeng[tbl % 2].dma_start(
                            out=out_d[tb * 128:(tb + 1) * 128, :], in_=osb[:]
                        )

                for qc in range(9):
                  if qc < 8:
                    q0 = qc * 512
                    kmax = 4 * (qc + 1)
                    ats = [psAT.tile([DH + 1, 512], F32, tag=f"atps{h}", name=f"at{h}")
                           for h in range(HL)]
                    for g in range((kmax + 1) // 2):
                        kbs = [kb for kb in (2 * g, 2 * g + 1) if kb < kmax]
                        sts = [psST.tile([128, 1024], F32, tag=f"stps{h}", name=f"st{h}")
                               for h in range(HL)]
                        # live column start within the chunk, per kb (0 for
                        # off-diagonal blocks, j*128 for diagonal block j)
                        lives = [max(0, (kb - 4 * qc) * 128) for kb in kbs]
                        # S matmuls: interleave heads so the two K=64
                        # matmuls occupy PE row groups 0-63 / 64-127.
                        for i, kb in enumerate(kbs):
                            lv = lives[i]
                            for h in range(HL):
                                hs = h * DH
                                nc.tensor.matmul(
                                    sts[h][:, i * 512 + lv:(i + 1) * 512],
                                    lhsT=kT[hs:hs + DH, kb * 128:(kb + 1) * 128],
                                    rhs=qT[hs:hs + DH, q0 + lv:q0 + 512],
                                    start=True, stop=True,
                                )
                        pts = []
                        for h in range(HL):
                            pt = ptpool.tile([128, 1024], BF16, tag=f"pt{h}")
                            if qc > 0 and max(lives) > 0:
                                for i, kb in enumerate(kbs):
                                    # exp only the live cols (dead cols are
                                    # zeroed by the full-width mask mul)
                                    lv = lives[i]
                                    nc.scalar.activation(
                                        pt[:, i * 512 + lv:(i + 1) * 512],
                                        sts[h][:, i * 512 + lv:(i + 1) * 512],
                                        mybir.ActivationFunctionType.Exp,
                                        scale=0.125,
                                    )
                            else:
                                nc.scalar.activation(
                                    pt[:], sts[h][:],
                                    mybir.ActivationFunctionType.Exp,
                                    scale=0.125,
                                )
                            pts.append(pt)
                        for i, kb in enumerate(kbs):
                            j = kb - 4 * qc
                            if j >= 0:
                                for h in range(HL):
                                    nc.vector.tensor_mul(
                                        pts[h][:, i * 512:(i + 1) * 512],
                                        pts[h][:, i * 512:(i + 1) * 512],
                                        dmask[:, j * 512:(j + 1) * 512],
                                    )
                        for i, kb in enumerate(kbs):
                            for h in range(HL):
                                o = kb * VBLK + h * 65
                                nc.tensor.matmul(
                                    ats[h][:],
                                    lhsT=vnat[:, o:o + 65],
                                    rhs=pts[h][:, i * 512:(i + 1) * 512],
                                    start=(kb == 0), stop=(kb == kmax - 1),
                                    skip_group_check=True,
                                )
                        if g == 0 and qc > 0:
                            # previous chunk's out-projection fills the
                            # exp pipeline warm-up stall of this chunk
                            emit_outp(qc - 1, rrow_prev)
                    # evacuate attn rows + sums; reshape the sums row
                    # [1, 512] -> [128, 4] with direct SBUF rearrange DMAs
                    # for a lane-parallel reciprocal (no DRAM bounce).
                    rrow = npool.tile([HL, 512], BF16, tag="rrow")
                    scomf = npool.tile([128, HL * 4], F32, tag="scomf")
                    rcomf = npool.tile([128, HL * 4], F32, tag="rcomf")
                    rcom = npool.tile([128, HL * 4], BF16, tag="rcom")
                    for h in range(HL):
                        hs = h * DH
                        ev = evpool.tile([DH + 1, 512], BF16, tag=f"ev{h}")
                        ssr = evpool.tile([DH + 1, 512], F32, tag=f"ssr{h}")
                        nc.vector.tensor_copy(ev[:], ats[h][:])
                        nc.sync.dma_start(
                            out=attnT[hs:hs + DH, q0:q0 + 512], in_=ev[0:DH, :]
                        )
                        nc.vector.tensor_copy(
                            ssr[DH:DH + 1, :], ats[h][DH:DH + 1, :])
                        nc.scalar.dma_start(
                            out=sums_d[h, q0:q0 + 512], in_=ssr[DH:DH + 1, :]
                        )
                        nc.scalar.dma_start(
                            out=scomf[:, h * 4:(h + 1) * 4],
                            in_=sums_d[h, q0:q0 + 512].rearrange(
                                "(p f) -> p f", p=128),
                        )
                    nc.vector.reciprocal(rcomf[:], scomf[:])
                    nc.vector.tensor_copy(rcom[:], rcomf[:])
                    for h in range(HL):
                        nc.scalar.dma_start(
                            out=recip_d[h, q0:q0 + 512].rearrange(
                                "(p f) -> p f", p=128),
                            in_=rcom[:, h * 4:(h + 1) * 4],
                        )
                    nc.scalar.dma_start(
                        out=rrow[:], in_=recip_d[:, q0:q0 + 512])
                    rrow_prev = rrow
                  else:
                    # final chunk's out-projection
                    emit_outp(7, rrow_prev)

    nc.compile()
    return nc


def _host_constants():
    import ml_dtypes
    bf = ml_dtypes.bfloat16
    inv_freq = 1.0 / (10000.0 ** (np.arange(0, DH, 2, dtype=np.float64) / DH))
    t = np.arange(T, dtype=np.float64)
    freqs = np.outer(t, inv_freq)  # [T, 32]
    emb = np.concatenate([freqs, freqs], axis=-1)  # [T, 64]
    cos = np.cos(emb).astype(np.float32).T  # [64, T]
    sin = np.sin(emb).astype(np.float32).T  # [64, T]
    sinS = sin.copy()
    sinS[0:DH // 2] *= -1.0  # fold rotate_half's negation into the table
    cos2 = np.ascontiguousarray(np.tile(cos, (HL, 1)))  # [128, T]
    sin2 = np.ascontiguousarray(np.tile(sinS, (HL, 1)))

    # swap-halves permutation (per 64-row head block), symmetric
    p1 = np.zeros((DH, DH), np.float32)
    half = DH // 2
    p1[np.arange(half), np.arange(half) + half] = 1.0
    p1[np.arange(half) + half, np.arange(half)] = 1.0
    p128 = np.block([
        [p1, np.zeros((DH, DH), np.float32)],
        [np.zeros((DH, DH), np.float32), p1],
    ]).astype(np.float32)

    ident = np.eye(128, dtype=np.float32)

    e2 = np.zeros((HL, 128), np.float32)
    for h in range(HL):
        e2[h, h * DH:(h + 1) * DH] = 1.0

    # diag masks [128, 4*512]: mask_j[k, q] = 1 iff q >= j*128 + k
    dmask = np.zeros((128, 4, 512), np.float32)
    kk = np.arange(128)[:, None]
    qq = np.arange(512)[None, :]
    for j in range(4):
        dmask[:, j, :] = (qq >= j * 128 + kk).astype(np.float32)
    dmask = np.ascontiguousarray(dmask.reshape(128, 4 * 512))

    return (cos2.astype(bf), sin2.astype(bf), p128.astype(bf),
            ident.astype(bf), e2.astype(bf), dmask.astype(bf))


_NC_CACHE = None


def _get_nc():
    global _NC_CACHE
    if _NC_CACHE is None:
        _NC_CACHE = build_nc()
    return _NC_CACHE


def _in_maps(x, W_qkv, W_out):
    import ml_dtypes
    bf = ml_dtypes.bfloat16
    x2 = np.asarray(x, np.float32).reshape(T, D)
    W_qkv = np.asarray(W_qkv, np.float32)
    W_out = np.asarray(W_out, np.float32)
    xT = np.ascontiguousarray(x2.T).astype(bf)
    cos2, sin2, p128, ident, e2, dmask = _host_constants()

    Wq, Wk, Wv = W_qkv[0:D], W_qkv[D:2 * D], W_qkv[2 * D:3 * D]
    in_maps = []
    for c in range(NC):
        h0, h1 = HL * c, HL * c + 1
        rows = []
        for Wp in (Wq, Wk, Wv):
            rows.append(Wp[h0 * DH:(h0 + 1) * DH])
            rows.append(Wp[h1 * DH:(h1 + 1) * DH])
        wqkvT = np.ascontiguousarray(
            np.concatenate(rows, axis=0).T).astype(bf)
        cols = np.r_[h0 * DH:(h0 + 1) * DH, h1 * DH:(h1 + 1) * DH]
        woutT = np.ascontiguousarray(W_out[:, cols].T).astype(bf)  # [128, D]
        in_maps.append({
            "xT": xT, "wqkvT": wqkvT, "woutT": woutT,
            "cos2": cos2, "sin2": sin2, "p128": p128,
            "ident": ident, "e2": e2, "dmask": dmask,
        })
    return in_maps


def _run(x, W_qkv, W_out, **spmd_kwargs):
    nc = _get_nc()
    res = run_bass_kernel_spmd(
        nc, _in_maps(x, W_qkv, W_out), core_ids=list(range(NC)), **spmd_kwargs
    )
    out = res.results[0]["outp"].astype(np.float64)
    for c in range(1, NC):
        out += res.results[c]["outp"].astype(np.float64)
    return out.astype(np.float32).reshape(1, T, D), res


def kernel(x, W_qkv, W_out):
    out, _ = _run(x, W_qkv, W_out)
    return out


# revision 19
# speedup vs baseline: 1.4232x; 1.0751x over previous
"""Causal self-attention (B=1, T=4096, D=1024, H=16, dh=64) on 8 trn2 NeuronCores.

Sharding: tensor-parallel over heads — each core owns 2 of the 16 heads.
Per core: QKV projection (transposed activation layout), RoPE, causal
flash-style attention with transposed score tiles St[k,q] (so the AV matmul
needs no P transposes), softmax denominator via an appended ones-column in V,
out-projection against this core's W_out column slice -> partial output
[T, D] in bf16.  Host sums the 8 partials in f32.

v3 (from the v1 structure, which empirically avoids sustained PE power
throttling thanks to its per-group exp stalls):
- all matmul operands bf16 (fp32r streamed at ~half PE rate on hw),
- diagonal score blocks compute only the live (unmasked) q-columns
  (-8% S/AV PE work, -12% exp work, less PE power),
- bf16 output partials (halves the output DMA; host sums in f32),
- bf16 softmax-sum bounce (half the scratch DMA),
- input/const DMAs spread across the sync/scalar/gpsimd queues.
"""

import sys

sys.path.insert(0, "/opt/trn_rl_repo")

import numpy as np

import concourse.bass as bass
import concourse.tile as tile
from concourse import bacc, mybir
from concourse.bass_utils import run_bass_kernel_spmd

T = 4096
D = 1024
H = 16
DH = 64
NC = 8
HL = H // NC  # heads per core (2)
DL = HL * DH  # local feature width (128)

F32 = mybir.dt.float32
BF16 = mybir.dt.bfloat16


def build_nc():
    nc = bacc.Bacc(
        "TRN2", target_bir_lowering=False, debug=False, num_devices=NC
    )

    # ---- DRAM I/O -------------------------------------------------------
    xT_d = nc.dram_tensor("xT", [D, T], BF16, kind="ExternalInput").ap()
    wqkvT_d = nc.dram_tensor("wqkvT", [D, 3 * DL], BF16, kind="ExternalInput").ap()
    woutT_d = nc.dram_tensor("woutT", [DL, D], BF16, kind="ExternalInput").ap()
    cos2_d = nc.dram_tensor("cos2", [DL, T], BF16, kind="ExternalInput").ap()
    sin2_d = nc.dram_tensor("sin2", [DL, T], BF16, kind="ExternalInput").ap()
    p128_d = nc.dram_tensor("p128", [DL, DL], BF16, kind="ExternalInput").ap()
    ident_d = nc.dram_tensor("ident", [128, 128], BF16, kind="ExternalInput").ap()
    ee_d = nc.dram_tensor("ee", [128, 2 * DL], F32, kind="ExternalInput").ap()
    # 4 diagonal-block masks [128, 512]: mask_j[k, q] = 1 iff q >= j*128 + k
    dmask_d = nc.dram_tensor("dmask", [128, 4 * 512], BF16, kind="ExternalInput").ap()
    out_d = nc.dram_tensor("outp", [T, D], BF16, kind="ExternalOutput").ap()

    NCH = 8  # T-chunks of 512 for the QKV projection
    CW = 512  # chunk width
    VBLK = 130  # v-nat block layout: [v_h0(64) | 1 | v_h1(64) | 1]

    with tile.TileContext(nc) as tc:
        with tc.tile_pool(name="consts", bufs=1) as cpool, \
             tc.tile_pool(name="persist", bufs=1) as ppool:
            # ---- constants needed immediately (QKV weights) ------------
            weng = [nc.sync, nc.gpsimd, nc.scalar]
            # first x chunk goes FIRST on every queue, interleaved with the
            # matching weight block, so QKV matmul d can start as soon as
            # (x_d, w_d) land.
            xt0 = None  # created below, before the weights
            wt = [cpool.tile([128, 3 * DL], BF16, tag=f"wt{d}", name=f"wt{d}")
                  for d in range(8)]
            p128 = cpool.tile([DL, DL], BF16, tag="p128")
            ident = cpool.tile([128, 128], BF16, tag="ident")

            # ---- persistent activations --------------------------------
            qT = ppool.tile([DL, T], BF16, tag="qT")
            kT = ppool.tile([DL, T], BF16, tag="kT")
            vnat = ppool.tile([128, (T // 128) * VBLK], BF16, tag="vnat")
            attnT = ppool.tile([DL, T], BF16, tag="attnT")

            # ones columns of the v-nat layout (cols 64 and 129 of each block)
            ones_sb = cpool.tile([128, T // 128], F32, tag="ones_sb")
            nc.gpsimd.memset(ones_sb[:], 1.0)
            vone = vnat[:].rearrange("p (b c) -> p b c", c=VBLK)
            nc.vector.tensor_copy(vone[:, :, 64:65], ones_sb[:].rearrange("p (b c) -> p b c", c=1))
            nc.vector.tensor_copy(vone[:, :, 129:130], ones_sb[:].rearrange("p (b c) -> p b c", c=1))

            # ---- remaining constants (deferred so x/W DMAs go first) ---
            cos2 = cpool.tile([DL, T], BF16, tag="cos2")
            sin2 = cpool.tile([DL, T], BF16, tag="sin2")
            woutT = cpool.tile([DL, D], BF16, tag="woutT")
            ee = cpool.tile([128, 2 * DL], F32, tag="ee")
            dmask = cpool.tile([128, 4 * 512], BF16, tag="dmask")

            # ================= Phase A: QKV + RoPE ======================
            with tc.tile_pool(name="xp", bufs=3) as xpool, \
                 tc.tile_pool(name="tmpa", bufs=3) as tpool, \
                 tc.tile_pool(name="psA", bufs=2, space="PSUM") as psA:
                xeng = [nc.sync, nc.scalar, nc.gpsimd]
                for c in range(NCH):
                    s = c * CW
                    xt = xpool.tile([128, 8 * CW], BF16, tag="xchunk")
                    for d in range(8):
                        xeng[d % 3].dma_start(
                            out=xt[:, d * CW:(d + 1) * CW],
                            in_=xT_d[d * 128:(d + 1) * 128, s:s + CW],
                        )
                        if c == 0:
                            # pair each first-chunk x block with its weight
                            weng[d % 3].dma_start(
                                out=wt[d][:],
                                in_=wqkvT_d[d * 128:(d + 1) * 128, :])
                    if c == 0:
                        # deferred constant loads, behind the first x chunk
                        nc.sync.dma_start(out=p128[:], in_=p128_d)
                        nc.gpsimd.dma_start(out=ident[:], in_=ident_d)
                        nc.gpsimd.dma_start(out=cos2[:], in_=cos2_d)
                        nc.gpsimd.dma_start(out=sin2[:], in_=sin2_d)
                        nc.sync.dma_start(out=dmask[:], in_=dmask_d)
                        nc.sync.dma_start(out=ee[:], in_=ee_d)
                        nc.scalar.dma_start(out=woutT[:], in_=woutT_d)

                    def xs(d):
                        return xt[:, d * CW:(d + 1) * CW]

                    # qT / kT with RoPE
                    for idx, dst in ((0, qT), (1, kT)):
                        pp = psA.tile([128, CW], F32, tag="qkvps")
                        for d in range(8):
                            nc.tensor.matmul(
                                pp[:],
                                lhsT=wt[d][:, idx * DL:(idx + 1) * DL],
                                rhs=xs(d),
                                start=(d == 0),
                                stop=(d == 7),
                            )
                        praw = tpool.tile([128, CW], BF16, tag="praw")
                        nc.vector.tensor_copy(praw[:], pp[:])
                        rot = psA.tile([128, CW], F32, tag="rotps")
                        nc.tensor.matmul(
                            rot[:], lhsT=p128[:], rhs=praw[:],
                            start=True, stop=True,
                        )
                        dstv = dst[:, s:s + CW]
                        nc.vector.tensor_mul(dstv, praw[:], cos2[:, s:s + CW])
                        rtmp = tpool.tile([128, CW], BF16, tag="rtmp")
                        nc.vector.tensor_mul(rtmp[:], rot[:], sin2[:, s:s + CW])
                        nc.gpsimd.tensor_add(dstv, dstv, rtmp[:])

                    # v: compute vT then PE-transpose to natural layout
                    vp = psA.tile([128, CW], F32, tag="qkvps")
                    for d in range(8):
                        nc.tensor.matmul(
                            vp[:], lhsT=wt[d][:, 2 * DL:3 * DL],
                            rhs=xs(d), start=(d == 0), stop=(d == 7),
                        )
                    vtmp = tpool.tile([128, CW], BF16, tag="vtmp")
                    nc.scalar.copy(vtmp[:], vp[:])
                    for b in range(CW // 128):
                        kb = (s // 128) + b
                        tp = psA.tile([128, 128], BF16, tag="vtps")
                        nc.tensor.transpose(
                            tp[:], vtmp[:, b * 128:(b + 1) * 128], ident[:]
                        )
                        o = kb * VBLK
                        nc.vector.tensor_copy(vnat[:, o:o + 64], tp[:, 0:64])
                        nc.vector.tensor_copy(vnat[:, o + 65:o + 129], tp[:, 64:128])

            # ====== Phase B+C: attention, normalize, out-projection =====
            # per q-chunk: both heads interleaved (their K=64 S-matmuls pack
            # into disjoint PE row groups), then normalize + project the
            # chunk's rows.  Diagonal blocks only compute live q-columns.
            with tc.tile_pool(name="ptp", bufs=2) as ptpool, \
                 tc.tile_pool(name="evp", bufs=3) as evpool, \
                 tc.tile_pool(name="nrm", bufs=2) as npool, \
                 tc.tile_pool(name="op", bufs=3) as opool, \
                 tc.tile_pool(name="psAT", bufs=1, space="PSUM") as psAT, \
                 tc.tile_pool(name="psST", bufs=1, space="PSUM") as psST, \
                 tc.tile_pool(name="psO", bufs=1, space="PSUM") as psO:
                rrow_prev = None
                oeng = [nc.sync, nc.gpsimd]

                def emit_outp(qcp, ssrs):
                    # normalize + project chunk qcp: broadcast each head's
                    # sums row (partition 64) to its 64 attnT rows with K=1
                    # matmuls, then one wide 128-lane reciprocal -- no DRAM
                    # round-trip.
                    qp0 = qcp * 512
                    rb = psO.tile([128, 512], F32, tag="rbps")
                    for h in range(HL):
                        nc.tensor.matmul(
                            rb[:],
                            lhsT=ee[64:65, h * DL:(h + 1) * DL],
                            rhs=ssrs[h][DH:DH + 1, :],
                            start=(h == 0), stop=(h == HL - 1),
                            skip_group_check=True,
                        )
                    rbR = npool.tile([128, 512], F32, tag="rbR")
                    with nc.allow_low_precision(reason="softmax recip"):
                        nc.vector.reciprocal(rbR[:], rb[:])
                    nc.vector.tensor_mul(
                        attnT[:, qp0:qp0 + 512], attnT[:, qp0:qp0 + 512],
                        rbR[:],
                    )
                    for tbl in range(4):
                        tb = qcp * 4 + tbl
                        osb = opool.tile([128, D], BF16, tag="osb")
                        for ec in range(2):
                            op = psO.tile([128, 512], F32, tag="ops")
                            nc.tensor.matmul(
                                op[:],
                                lhsT=attnT[:, tb * 128:(tb + 1) * 128],
                                rhs=woutT[:, ec * 512:(ec + 1) * 512],
                                start=True, stop=True,
                                skip_group_check=True,
                            )
                            nc.vector.tensor_copy(
                                osb[:, ec * 512:(ec + 1) * 512], op[:])
                        oeng[tbl % 2].dma_start(
                            out=out_d[tb * 128:(tb + 1) * 128, :], in_=osb[:]
                        )

                for qc in range(9):
                  if qc < 8:
                    q0 = qc * 512
                    kmax = 4 * (qc + 1)
                    ats = [psAT.tile([DH + 1, 512], F32, tag=f"atps{h}", name=f"at{h}")
                           for h in range(HL)]
                    for g in range((kmax + 1) // 2):
                        kbs = [kb for kb in (2 * g, 2 * g + 1) if kb < kmax]
                        sts = [psST.tile([128, 1024], F32, tag=f"stps{h}", name=f"st{h}")
                               for h in range(HL)]
                        # live column start within the chunk, per kb (0 for
                        # off-diagonal blocks, j*128 for diagonal block j)
                        lives = [max(0, (kb - 4 * qc) * 128) for kb in kbs]
                        # S matmuls: interleave heads so the two K=64
                        # matmuls occupy PE row groups 0-63 / 64-127.
                        for i, kb in enumerate(kbs):
                            lv = lives[i]
                            for h in range(HL):
                                hs = h * DH
                                nc.tensor.matmul(
                                    sts[h][:, i * 512 + lv:(i + 1) * 512],
                                    lhsT=kT[hs:hs + DH, kb * 128:(kb + 1) * 128],
                                    rhs=qT[hs:hs + DH, q0 + lv:q0 + 512],
                                    start=True, stop=True,
                                )
                        pts = []
                        for h in range(HL):
                            pt = ptpool.tile([128, 1024], BF16, tag=f"pt{h}")
                            if qc > 0 and max(lives) > 0:
                                for i, kb in enumerate(kbs):
                                    # exp only the live cols (dead cols are
                                    # zeroed by the full-width mask mul)
                                    lv = lives[i]
                                    nc.scalar.activation(
                                        pt[:, i * 512 + lv:(i + 1) * 512],
                                        sts[h][:, i * 512 + lv:(i + 1) * 512],
                                        mybir.ActivationFunctionType.Exp,
                                        scale=0.125,
                                    )
                            else:
                                nc.scalar.activation(
                                    pt[:], sts[h][:],
                                    mybir.ActivationFunctionType.Exp,
                                    scale=0.125,
                                )
                            pts.append(pt)
                        for i, kb in enumerate(kbs):
                            j = kb - 4 * qc
                            if j >= 0:
                                for h in range(HL):
                                    nc.vector.tensor_mul(
                                        pts[h][:, i * 512:(i + 1) * 512],
                                        pts[h][:, i * 512:(i + 1) * 512],
                                        dmask[:, j * 512:(j + 1) * 512],
                                    )
                        for i, kb in enumerate(kbs):
                            for h in range(HL):
                                o = kb * VBLK + h * 65
                                nc.tensor.matmul(
                                    ats[h][:],
                                    lhsT=vnat[:, o:o + 65],
                                    rhs=pts[h][:, i * 512:(i + 1) * 512],
                                    start=(kb == 0), stop=(kb == kmax - 1),
                                    skip_group_check=True,
                                )
                        if g == 0 and qc > 0:
                            # previous chunk's out-projection fills the
                            # exp pipeline warm-up stall of this chunk
                            emit_outp(qc - 1, rrow_prev)
                    # evacuate attn rows and the sums row (kept in-lane
                    # at partition 64; broadcast happens in emit_outp)
                    ssrs = []
                    for h in range(HL):
                        hs = h * DH
                        ev = evpool.tile([DH + 1, 512], BF16, tag=f"ev{h}")
                        ssr = evpool.tile([DH + 1, 512], F32, tag=f"ssr{h}")
                        nc.vector.tensor_copy(ev[:], ats[h][:])
                        nc.sync.dma_start(
                            out=attnT[hs:hs + DH, q0:q0 + 512], in_=ev[0:DH, :]
                        )
                        nc.vector.tensor_copy(
                            ssr[DH:DH + 1, :], ats[h][DH:DH + 1, :])
                        ssrs.append(ssr)
                    rrow_prev = ssrs
                  else:
                    # final chunk's out-projection
                    emit_outp(7, rrow_prev)

    nc.compile()
    return nc


def _host_constants():
    import ml_dtypes
    bf = ml_dtypes.bfloat16
    inv_freq = 1.0 / (10000.0 ** (np.arange(0, DH, 2, dtype=np.float64) / DH))
    t = np.arange(T, dtype=np.float64)
    freqs = np.outer(t, inv_freq)  # [T, 32]
    emb = np.concatenate([freqs, freqs], axis=-1)  # [T, 64]
    cos = np.cos(emb).astype(np.float32).T  # [64, T]
    sin = np.sin(emb).astype(np.float32).T  # [64, T]
    sinS = sin.copy()
    sinS[0:DH // 2] *= -1.0  # fold rotate_half's negation into the table
    cos2 = np.ascontiguousarray(np.tile(cos, (HL, 1)))  # [128, T]
    sin2 = np.ascontiguousarray(np.tile(sinS, (HL, 1)))

    # swap-halves permutation (per 64-row head block), symmetric
    p1 = np.zeros((DH, DH), np.float32)
    half = DH // 2
    p1[np.arange(half), np.arange(half) + half] = 1.0
    p1[np.arange(half) + half, np.arange(half)] = 1.0
    p128 = np.block([
        [p1, np.zeros((DH, DH), np.float32)],
        [np.zeros((DH, DH), np.float32), p1],
    ]).astype(np.float32)

    ident = np.eye(128, dtype=np.float32)

    # ee[:, h*128 + h*64 : ...] = e_h (one-hot 64-block), all partitions
    ee = np.zeros((128, 2 * DL), np.float32)
    for h in range(HL):
        ee[:, h * DL + h * DH:h * DL + (h + 1) * DH] = 1.0

    # diag masks [128, 4*512]: mask_j[k, q] = 1 iff q >= j*128 + k
    dmask = np.zeros((128, 4, 512), np.float32)
    kk = np.arange(128)[:, None]
    qq = np.arange(512)[None, :]
    for j in range(4):
        dmask[:, j, :] = (qq >= j * 128 + kk).astype(np.float32)
    dmask = np.ascontiguousarray(dmask.reshape(128, 4 * 512))

    return (cos2.astype(bf), sin2.astype(bf), p128.astype(bf),
            ident.astype(bf), ee, dmask.astype(bf))


_NC_CACHE = None


def _get_nc():
    global _NC_CACHE
    if _NC_CACHE is None:
        _NC_CACHE = build_nc()
    return _NC_CACHE


def _in_maps(x, W_qkv, W_out):
    import ml_dtypes
    bf = ml_dtypes.bfloat16
    x2 = np.asarray(x, np.float32).reshape(T, D)
    W_qkv = np.asarray(W_qkv, np.float32)
    W_out = np.asarray(W_out, np.float32)
    xT = np.ascontiguousarray(x2.T).astype(bf)
    cos2, sin2, p128, ident, ee, dmask = _host_constants()

    Wq, Wk, Wv = W_qkv[0:D], W_qkv[D:2 * D], W_qkv[2 * D:3 * D]
    in_maps = []
    for c in range(NC):
        h0, h1 = HL * c, HL * c + 1
        rows = []
        for Wp in (Wq, Wk, Wv):
            rows.append(Wp[h0 * DH:(h0 + 1) * DH])
            rows.append(Wp[h1 * DH:(h1 + 1) * DH])
        wqkvT = np.ascontiguousarray(
            np.concatenate(rows, axis=0).T).astype(bf)
        cols = np.r_[h0 * DH:(h0 + 1) * DH, h1 * DH:(h1 + 1) * DH]
        woutT = np.ascontiguousarray(W_out[:, cols].T).astype(bf)  # [128, D]
        in_maps.append({
            "xT": xT, "wqkvT": wqkvT, "woutT": woutT,
            "cos2": cos2, "sin2": sin2, "p128": p128,
            "ident": ident, "ee": ee, "dmask": dmask,
        })
    return in_maps


def _run(x, W_qkv, W_out, **spmd_kwargs):
    nc = _get_nc()
    res = run_bass_kernel_spmd(
        nc, _in_maps(x, W_qkv, W_out), core_ids=list(range(NC)), **spmd_kwargs
    )
    out = res.results[0]["outp"].astype(np.float64)
    for c in range(1, NC):
        out += res.results[c]["outp"].astype(np.float64)
    return out.astype(np.float32).reshape(1, T, D), res


def kernel(x, W_qkv, W_out):
    out, _ = _run(x, W_qkv, W_out)
    return out


# revision 20
# speedup vs baseline: 1.5727x; 1.1050x over previous
"""Causal self-attention (B=1, T=4096, D=1024, H=16, dh=64) on 8 trn2 NeuronCores.

Sharding: tensor-parallel over heads — each core owns 2 of the 16 heads.
Per core: QKV projection (transposed activation layout), RoPE, causal
flash-style attention with transposed score tiles St[k,q] (so the AV matmul
needs no P transposes), softmax denominator via an appended ones-column in V,
out-projection against this core's W_out column slice -> partial output
[T, D] in bf16.  Host sums the 8 partials in f32.

v3 (from the v1 structure, which empirically avoids sustained PE power
throttling thanks to its per-group exp stalls):
- all matmul operands bf16 (fp32r streamed at ~half PE rate on hw),
- diagonal score blocks compute only the live (unmasked) q-columns
  (-8% S/AV PE work, -12% exp work, less PE power),
- bf16 output partials (halves the output DMA; host sums in f32),
- bf16 softmax-sum bounce (half the scratch DMA),
- input/const DMAs spread across the sync/scalar/gpsimd queues.
"""

import sys

sys.path.insert(0, "/opt/trn_rl_repo")

import numpy as np

import concourse.bass as bass
import concourse.tile as tile
from concourse import bacc, mybir
from concourse.bass_utils import run_bass_kernel_spmd

T = 4096
D = 1024
H = 16
DH = 64
NC = 8
HL = H // NC  # heads per core (2)
DL = HL * DH  # local feature width (128)

F32 = mybir.dt.float32
BF16 = mybir.dt.bfloat16


def build_nc():
    nc = bacc.Bacc(
        "TRN2", target_bir_lowering=False, debug=False, num_devices=NC
    )

    # ---- DRAM I/O -------------------------------------------------------
    xT_d = nc.dram_tensor("xT", [D, T], BF16, kind="ExternalInput").ap()
    wqkvT_d = nc.dram_tensor("wqkvT", [D, 3 * DL], BF16, kind="ExternalInput").ap()
    woutT_d = nc.dram_tensor("woutT", [DL, D], BF16, kind="ExternalInput").ap()
    cos2_d = nc.dram_tensor("cos2", [DL, T], BF16, kind="ExternalInput").ap()
    sin2_d = nc.dram_tensor("sin2", [DL, T], BF16, kind="ExternalInput").ap()
    p128_d = nc.dram_tensor("p128", [DL, DL], BF16, kind="ExternalInput").ap()
    ident_d = nc.dram_tensor("ident", [128, 128], BF16, kind="ExternalInput").ap()
    ee_d = nc.dram_tensor("ee", [128, 2 * DL], F32, kind="ExternalInput").ap()
    # 4 diagonal-block masks [128, 512]: mask_j[k, q] = 1 iff q >= j*128 + k
    dmask_d = nc.dram_tensor("dmask", [128, 4 * 512], BF16, kind="ExternalInput").ap()
    out_d = nc.dram_tensor("outp", [T, D], BF16, kind="ExternalOutput").ap()

    NCH = 8  # T-chunks of 512 for the QKV projection
    CW = 512  # chunk width
    VBLK = 130  # v-nat block layout: [v_h0(64) | 1 | v_h1(64) | 1]

    with tile.TileContext(nc) as tc:
        with tc.tile_pool(name="consts", bufs=1) as cpool, \
             tc.tile_pool(name="persist", bufs=1) as ppool:
            # ---- constants needed immediately (QKV weights) ------------
            weng = [nc.sync, nc.gpsimd, nc.scalar]
            # first x chunk goes FIRST on every queue, interleaved with the
            # matching weight block, so QKV matmul d can start as soon as
            # (x_d, w_d) land.
            xt0 = None  # created below, before the weights
            wt = [cpool.tile([128, 3 * DL], BF16, tag=f"wt{d}", name=f"wt{d}")
                  for d in range(8)]
            p128 = cpool.tile([DL, DL], BF16, tag="p128")
            ident = cpool.tile([128, 128], BF16, tag="ident")

            # ---- persistent activations --------------------------------
            qT = ppool.tile([DL, T], BF16, tag="qT")
            kT = ppool.tile([DL, T], BF16, tag="kT")
            vnat = ppool.tile([128, (T // 128) * VBLK], BF16, tag="vnat")
            attnT = ppool.tile([DL, T], BF16, tag="attnT")

            # ones columns of the v-nat layout (cols 64 and 129 of each block)
            ones_sb = cpool.tile([128, T // 128], F32, tag="ones_sb")
            nc.gpsimd.memset(ones_sb[:], 1.0)
            vone = vnat[:].rearrange("p (b c) -> p b c", c=VBLK)
            nc.vector.tensor_copy(vone[:, :, 64:65], ones_sb[:].rearrange("p (b c) -> p b c", c=1))
            nc.vector.tensor_copy(vone[:, :, 129:130], ones_sb[:].rearrange("p (b c) -> p b c", c=1))

            # ---- remaining constants (deferred so x/W DMAs go first) ---
            cos2 = cpool.tile([DL, T], BF16, tag="cos2")
            sin2 = cpool.tile([DL, T], BF16, tag="sin2")
            woutT = cpool.tile([DL, D], BF16, tag="woutT")
            ee = cpool.tile([128, 2 * DL], F32, tag="ee")
            dmask = cpool.tile([128, 4 * 512], BF16, tag="dmask")

            # ================= Phase A: QKV + RoPE ======================
            with tc.tile_pool(name="xp", bufs=3) as xpool, \
                 tc.tile_pool(name="tmpa", bufs=3) as tpool, \
                 tc.tile_pool(name="psA", bufs=2, space="PSUM") as psA:
                xeng = [nc.sync, nc.scalar, nc.gpsimd]
                for c in range(NCH):
                    s = c * CW
                    xt = xpool.tile([128, 8 * CW], BF16, tag="xchunk")
                    for d in range(8):
                        xeng[d % 3].dma_start(
                            out=xt[:, d * CW:(d + 1) * CW],
                            in_=xT_d[d * 128:(d + 1) * 128, s:s + CW],
                        )
                        if c == 0:
                            # pair each first-chunk x block with its weight
                            weng[d % 3].dma_start(
                                out=wt[d][:],
                                in_=wqkvT_d[d * 128:(d + 1) * 128, :])
                    if c == 0:
                        # deferred constant loads, behind the first x chunk
                        nc.sync.dma_start(out=p128[:], in_=p128_d)
                        nc.gpsimd.dma_start(out=ident[:], in_=ident_d)
                        nc.gpsimd.dma_start(out=cos2[:], in_=cos2_d)
                        nc.gpsimd.dma_start(out=sin2[:], in_=sin2_d)
                        nc.sync.dma_start(out=dmask[:], in_=dmask_d)
                        nc.sync.dma_start(out=ee[:], in_=ee_d)
                        nc.scalar.dma_start(out=woutT[:], in_=woutT_d)

                    def xs(d):
                        return xt[:, d * CW:(d + 1) * CW]

                    # qT / kT with RoPE
                    for idx, dst in ((0, qT), (1, kT)):
                        pp = psA.tile([128, CW], F32, tag="qkvps")
                        for d in range(8):
                            nc.tensor.matmul(
                                pp[:],
                                lhsT=wt[d][:, idx * DL:(idx + 1) * DL],
                                rhs=xs(d),
                                start=(d == 0),
                                stop=(d == 7),
                            )
                        praw = tpool.tile([128, CW], BF16, tag="praw")
                        nc.vector.tensor_copy(praw[:], pp[:])
                        rot = psA.tile([128, CW], F32, tag="rotps")
                        nc.tensor.matmul(
                            rot[:], lhsT=p128[:], rhs=praw[:],
                            start=True, stop=True,
                        )
                        dstv = dst[:, s:s + CW]
                        nc.vector.tensor_mul(dstv, praw[:], cos2[:, s:s + CW])
                        rtmp = tpool.tile([128, CW], BF16, tag="rtmp")
                        nc.vector.tensor_mul(rtmp[:], rot[:], sin2[:, s:s + CW])
                        nc.gpsimd.tensor_add(dstv, dstv, rtmp[:])

                    # v: compute vT then PE-transpose to natural layout
                    vp = psA.tile([128, CW], F32, tag="qkvps")
                    for d in range(8):
                        nc.tensor.matmul(
                            vp[:], lhsT=wt[d][:, 2 * DL:3 * DL],
                            rhs=xs(d), start=(d == 0), stop=(d == 7),
                        )
                    vtmp = tpool.tile([128, CW], BF16, tag="vtmp")
                    nc.scalar.copy(vtmp[:], vp[:])
                    for b in range(CW // 128):
                        kb = (s // 128) + b
                        tp = psA.tile([128, 128], BF16, tag="vtps")
                        nc.tensor.transpose(
                            tp[:], vtmp[:, b * 128:(b + 1) * 128], ident[:]
                        )
                        o = kb * VBLK
                        nc.vector.tensor_copy(vnat[:, o:o + 64], tp[:, 0:64])
                        nc.vector.tensor_copy(vnat[:, o + 65:o + 129], tp[:, 64:128])

            # ====== Phase B+C: attention, normalize, out-projection =====
            # per q-chunk: both heads interleaved (their K=64 S-matmuls pack
            # into disjoint PE row groups), then normalize + project the
            # chunk's rows.  Diagonal blocks only compute live q-columns.
            with tc.tile_pool(name="ptp", bufs=2) as ptpool, \
                 tc.tile_pool(name="evp", bufs=3) as evpool, \
                 tc.tile_pool(name="nrm", bufs=2) as npool, \
                 tc.tile_pool(name="op", bufs=3) as opool, \
                 tc.tile_pool(name="psAT", bufs=1, space="PSUM") as psAT, \
                 tc.tile_pool(name="psST", bufs=1, space="PSUM") as psST, \
                 tc.tile_pool(name="psO", bufs=1, space="PSUM") as psO:
                rrow_prev = None
                oeng = [nc.sync, nc.gpsimd]

                def emit_outp(qcp, ssrs):
                    # normalize + project chunk qcp: broadcast each head's
                    # sums row (partition 64) to its 64 attnT rows with K=1
                    # matmuls, then one wide 128-lane reciprocal -- no DRAM
                    # round-trip.
                    qp0 = qcp * 512
                    rb = psO.tile([128, 512], F32, tag="rbps")
                    for h in range(HL):
                        nc.tensor.matmul(
                            rb[:],
                            lhsT=ee[64:65, h * DL:(h + 1) * DL],
                            rhs=ssrs[h][DH:DH + 1, :],
                            start=(h == 0), stop=(h == HL - 1),
                            skip_group_check=True,
                        )
                    rbR = npool.tile([128, 512], F32, tag="rbR")
                    with nc.allow_low_precision(reason="softmax recip"):
                        nc.vector.reciprocal(rbR[:], rb[:])
                    nc.vector.tensor_mul(
                        attnT[:, qp0:qp0 + 512], attnT[:, qp0:qp0 + 512],
                        rbR[:],
                    )
                    for tbl in range(4):
                        tb = qcp * 4 + tbl
                        osb = opool.tile([128, D], BF16, tag="osb")
                        for ec in range(2):
                            op = psO.tile([128, 512], F32, tag="ops")
                            nc.tensor.matmul(
                                op[:],
                                lhsT=attnT[:, tb * 128:(tb + 1) * 128],
                                rhs=woutT[:, ec * 512:(ec + 1) * 512],
                                start=True, stop=True,
                                skip_group_check=True,
                            )
                            nc.vector.tensor_copy(
                                osb[:, ec * 512:(ec + 1) * 512], op[:])
                        oeng[tbl % 2].dma_start(
                            out=out_d[tb * 128:(tb + 1) * 128, :], in_=osb[:]
                        )

                for qc in range(9):
                  if qc < 8:
                    q0 = qc * 512
                    kmax = 4 * (qc + 1)
                    ats = [psAT.tile([DH + 1, 512], F32, tag=f"atps{h}", name=f"at{h}")
                           for h in range(HL)]
                    for g in range((kmax + 1) // 2):
                        kbs = [kb for kb in (2 * g, 2 * g + 1) if kb < kmax]
                        sts = [psST.tile([128, 1024], F32, tag=f"stps{h}", name=f"st{h}")
                               for h in range(HL)]
                        # live column start within the chunk, per kb (0 for
                        # off-diagonal blocks, j*128 for diagonal block j)
                        lives = [max(0, (kb - 4 * qc) * 128) for kb in kbs]
                        # S matmuls: interleave heads so the two K=64
                        # matmuls occupy PE row groups 0-63 / 64-127.
                        for i, kb in enumerate(kbs):
                            lv = lives[i]
                            for h in range(HL):
                                hs = h * DH
                                nc.tensor.matmul(
                                    sts[h][:, i * 512 + lv:(i + 1) * 512],
                                    lhsT=kT[hs:hs + DH, kb * 128:(kb + 1) * 128],
                                    rhs=qT[hs:hs + DH, q0 + lv:q0 + 512],
                                    start=True, stop=True,
                                )
                        pts = []
                        for h in range(HL):
                            pt = ptpool.tile([128, 1024], BF16, tag=f"pt{h}")
                            if max(lives) > 0:
                                for i, kb in enumerate(kbs):
                                    # exp/mask/AV all restricted to the live
                                    # cols; dead cols are never read (kb==0
                                    # is always full-width, so the ats PSUM
                                    # init covers every column)
                                    lv = lives[i]
                                    nc.scalar.activation(
                                        pt[:, i * 512 + lv:(i + 1) * 512],
                                        sts[h][:, i * 512 + lv:(i + 1) * 512],
                                        mybir.ActivationFunctionType.Exp,
                                        scale=0.125,
                                    )
                            else:
                                nc.scalar.activation(
                                    pt[:], sts[h][:],
                                    mybir.ActivationFunctionType.Exp,
                                    scale=0.125,
                                )
                            pts.append(pt)
                        for i, kb in enumerate(kbs):
                            j = kb - 4 * qc
                            if j >= 0:
                                lv = lives[i]
                                for h in range(HL):
                                    nc.vector.tensor_mul(
                                        pts[h][:, i * 512 + lv:(i + 1) * 512],
                                        pts[h][:, i * 512 + lv:(i + 1) * 512],
                                        dmask[:, j * 512 + lv:(j + 1) * 512],
                                    )
                        for i, kb in enumerate(kbs):
                            lv = lives[i]
                            for h in range(HL):
                                o = kb * VBLK + h * 65
                                nc.tensor.matmul(
                                    ats[h][:, lv:512],
                                    lhsT=vnat[:, o:o + 65],
                                    rhs=pts[h][:, i * 512 + lv:(i + 1) * 512],
                                    start=(kb == 0), stop=(kb == kmax - 1),
                                    skip_group_check=True,
                                )
                        if g == 0 and qc > 0:
                            # previous chunk's out-projection fills the
                            # exp pipeline warm-up stall of this chunk
                            emit_outp(qc - 1, rrow_prev)
                    # evacuate attn rows and the sums row (kept in-lane
                    # at partition 64; broadcast happens in emit_outp)
                    ssrs = []
                    for h in range(HL):
                        hs = h * DH
                        ev = evpool.tile([DH + 1, 512], BF16, tag=f"ev{h}")
                        ssr = evpool.tile([DH + 1, 512], F32, tag=f"ssr{h}")
                        nc.vector.tensor_copy(
                            ssr[DH:DH + 1, :], ats[h][DH:DH + 1, :])
                        nc.vector.tensor_copy(ev[:], ats[h][:])
                        nc.sync.dma_start(
                            out=attnT[hs:hs + DH, q0:q0 + 512], in_=ev[0:DH, :]
                        )
                        ssrs.append(ssr)
                    rrow_prev = ssrs
                  else:
                    # final chunk's out-projection
                    emit_outp(7, rrow_prev)

    nc.compile()
    return nc


def _host_constants():
    import ml_dtypes
    bf = ml_dtypes.bfloat16
    inv_freq = 1.0 / (10000.0 ** (np.arange(0, DH, 2, dtype=np.float64) / DH))
    t = np.arange(T, dtype=np.float64)
    freqs = np.outer(t, inv_freq)  # [T, 32]
    emb = np.concatenate([freqs, freqs], axis=-1)  # [T, 64]
    cos = np.cos(emb).astype(np.float32).T  # [64, T]
    sin = np.sin(emb).astype(np.float32).T  # [64, T]
    sinS = sin.copy()
    sinS[0:DH // 2] *= -1.0  # fold rotate_half's negation into the table
    cos2 = np.ascontiguousarray(np.tile(cos, (HL, 1)))  # [128, T]
    sin2 = np.ascontiguousarray(np.tile(sinS, (HL, 1)))

    # swap-halves permutation (per 64-row head block), symmetric
    p1 = np.zeros((DH, DH), np.float32)
    half = DH // 2
    p1[np.arange(half), np.arange(half) + half] = 1.0
    p1[np.arange(half) + half, np.arange(half)] = 1.0
    p128 = np.block([
        [p1, np.zeros((DH, DH), np.float32)],
        [np.zeros((DH, DH), np.float32), p1],
    ]).astype(np.float32)

    ident = np.eye(128, dtype=np.float32)

    # ee[:, h*128 + h*64 : ...] = e_h (one-hot 64-block), all partitions
    ee = np.zeros((128, 2 * DL), np.float32)
    for h in range(HL):
        ee[:, h * DL + h * DH:h * DL + (h + 1) * DH] = 1.0

    # diag masks [128, 4*512]: mask_j[k, q] = 1 iff q >= j*128 + k
    dmask = np.zeros((128, 4, 512), np.float32)
    kk = np.arange(128)[:, None]
    qq = np.arange(512)[None, :]
    for j in range(4):
        dmask[:, j, :] = (qq >= j * 128 + kk).astype(np.float32)
    dmask = np.ascontiguousarray(dmask.reshape(128, 4 * 512))

    return (cos2.astype(bf), sin2.astype(bf), p128.astype(bf),
            ident.astype(bf), ee, dmask.astype(bf))


_NC_CACHE = None


def _get_nc():
    global _NC_CACHE
    if _NC_CACHE is None:
        _NC_CACHE = build_nc()
    return _NC_CACHE


def _in_maps(x, W_qkv, W_out):
    import ml_dtypes
    bf = ml_dtypes.bfloat16
    x2 = np.asarray(x, np.float32).reshape(T, D)
    W_qkv = np.asarray(W_qkv, np.float32)
    W_out = np.asarray(W_out, np.float32)
    xT = np.ascontiguousarray(x2.T).astype(bf)
    cos2, sin2, p128, ident, ee, dmask = _host_constants()

    Wq, Wk, Wv = W_qkv[0:D], W_qkv[D:2 * D], W_qkv[2 * D:3 * D]
    in_maps = []
    for c in range(NC):
        h0, h1 = HL * c, HL * c + 1
        rows = []
        for Wp in (Wq, Wk, Wv):
            rows.append(Wp[h0 * DH:(h0 + 1) * DH])
            rows.append(Wp[h1 * DH:(h1 + 1) * DH])
        wqkvT = np.ascontiguousarray(
            np.concatenate(rows, axis=0).T).astype(bf)
        cols = np.r_[h0 * DH:(h0 + 1) * DH, h1 * DH:(h1 + 1) * DH]
        woutT = np.ascontiguousarray(W_out[:, cols].T).astype(bf)  # [128, D]
        in_maps.append({
            "xT": xT, "wqkvT": wqkvT, "woutT": woutT,
            "cos2": cos2, "sin2": sin2, "p128": p128,
            "ident": ident, "ee": ee, "dmask": dmask,
        })
    return in_maps


def _run(x, W_qkv, W_out, **spmd_kwargs):
    nc = _get_nc()
    res = run_bass_kernel_spmd(
        nc, _in_maps(x, W_qkv, W_out), core_ids=list(range(NC)), **spmd_kwargs
    )
    out = res.results[0]["outp"].astype(np.float64)
    for c in range(1, NC):
        out += res.results[c]["outp"].astype(np.float64)
    return out.astype(np.float32).reshape(1, T, D), res


def kernel(x, W_qkv, W_out):
    out, _ = _run(x, W_qkv, W_out)
    return out
